# revision 52
# baseline (speedup 1.0000x reference)
"""Trainium2 Bass kernel for nn_BasicClassifier (spiking conv classifier).

Sharding: pure data parallelism — batch 256 is split 32 samples per core
across 8 NeuronCores; params are replicated (tiny).

Per-core design (~154us). The T=1000 LIF scan is sequential; everything is
column-ordered PAIR-INTERLEAVED by (super-tick s, sample c, parity e) with
tick = 2s+e so the DVE can run a custom 2X_1PORT "double-step" op:

  - State ring: fp16 [128, 2048] tiles (x4). Ring col s*256 + 2i + e holds
    membrane i at tick 2s+e (i<96: layer-1 feature g*32+c; i>=96: layer-2
    unit row x sample col), lagged SKEW=64 ticks for layer 2.
  - LIF_DBL_ANT59: hand-written 2X_1PORT uOp program computing TWO LIF
    steps  m' = (m*0.9 + c) - (m > 1)  per cycle-column: reads the in0
    pair (HI = prev odd-tick state), the in1 drive pair (c_even, c_odd),
    chains both steps through the 8 ALU stages, writes the (m_e, m_o) pair
    via WR0_LO/HI. Recurrence lag = 128 pairs = 128 cycles of write->read
    distance (measured safe; 64 cycles is NOT — see pitch sweep). Two ops
    per 16-tick block: a 1-super bridge + a fused op whose in0 is its own
    out shifted one super back. perf_max=1 must be stamped on the FINAL
    instruction list (the Tile scheduler re-emits instructions).
  - PE: 3 conv matmuls (bf16 [xh;xl;xh]x[wh;wh;wl] K-stacks, exact to
    ~2^-16) + 3 fc matmuls (fp16 weights over sigma=(m1>=1) in {0,1},
    start=True on g0) per block. No bias matmul anywhere.
  - ACT: ONE merged 4-bank psum drain into the fp16 drive tile with the fc
    bias added via the per-partition bias port; the conv ones-row weights
    pre-subtract fcb[f%128] host-side so the m1 banks stay exact.
  - DVE stock tensor_scalar(is_ge) makes sigma (runs the fw 2x/4x modes).
  - fc sigma lead = 2 blocks (SKEW=4*BLK) so fc matmuls never wait on the
    freshest DVE block.
  - mem2 history: DMA from ring cols (s,192:256) via the POOL engine's DMA
    queues (latency-critical ring WAR; keeps them off the x-window queue).
    Host sums over (s, e).
"""

import os
import sys

for _p in ("/opt/trn_rl_repo", "/opt/pypackages"):
    if _p not in sys.path:
        sys.path.insert(0, _p)

import numpy as np

import concourse.bacc as bacc
import concourse.mybir as mybir
import concourse.tile as tile
import concourse.dve_ops as dve_ops
from concourse.dve_spec import Spec, Src0, Src1, C0, C1, lower
from concourse.dve_uop import (
    AluInp,
    AluOp,
    DelayInp,
    DveOpSpec,
    InpSel,
    OutPath,
    OutSel,
    Trigger,
    UopConfig,
)
from concourse.bass_utils import run_bass_kernel_spmd

F32 = mybir.dt.float32
F16 = mybir.dt.float16
BF16 = mybir.dt.bfloat16
ALU = mybir.AluOpType
AF = mybir.ActivationFunctionType

N_CORES = 8
B_FULL, T_FULL, L_IN = 256, 1000, 30
BC = B_FULL // N_CORES      # 32 samples per core
CH, LO = 16, 24
F = CH * LO                 # 384 features
G = 3                       # feature groups of 128
J = 35                      # fc outputs
KX = L_IN + 1               # conv contraction rows (30 taps + ones row)
BLK = 16                    # ticks per block (= 4 PSUM banks of drive)
SKEW = 5 * BLK              # layer-2 lag: spikes at tick t drive m2 at t+SKEW
WIN = 160                   # ticks per x-window DMA (multiple of BLK)
BETA, THR = 0.9, 1.0

TRACE = bool(int(os.environ.get("KERNEL_TRACE", "0")))
LIF2X = bool(int(os.environ.get("KERNEL_LIF2X", "1")))
LAST_RESULTS = None

_LIF_OP = None
_DBL_OP = None


def _lif_dbl_2x_uop():
    """2X_1PORT uOp program computing a DOUBLE LIF step per cycle.

    State is stored pair-interleaved: ring col s*256 + 2i + e holds membrane
    i at tick 2s+e. Each cycle the engine reads one in0 pair (only the HI
    element m = tick 2s-1 state is used), one in1 pair (c_even, c_odd), and
    the 8 ALU stages chain two full LIF steps, emitting the (m_even, m_odd)
    pair via WR0_LO/WR0_HI. The recurrence lag is one super-tick = 128
    pairs = 128 cycles of write->read distance (the same margin the proven
    1x single-step design has)."""
    u = UopConfig()
    u.enable_input(InpSel.SRC_0_HI, 1)   # d0 = m (prev odd-tick state)
    u.enable_input(InpSel.SRC_1, 2)      # d1 = c_even
    u.enable_input(InpSel.SRC_1_HI, 3)   # d2 = c_odd
    u.enable_input(InpSel.CONST_0, 4)    # d3 = beta
    u.enable_input(InpSel.CONST_1, 5)    # d4 = thr
    u.require_inp0 = 1
    u.require_inp1 = 1
    u.trigger = (Trigger.SRC_TENSOR_DONE, Trigger.NONE, Trigger.NONE)
    u.next_uop = (0, 0, 0)
    dp = u.datapath_config
    # S0: alu = m*beta
    dp[0].enable_alu(AluOp.MULTIPLY, AluInp.PREV_DELAY_0, AluInp.PREV_DELAY_3
                     ).pass_through_delay(0, 1, 2, 3, 4)
    # S1: alu = f_a = m*beta + c_even
    dp[1].enable_alu(AluOp.ADD, AluInp.PREV_ALU_OUT, AluInp.PREV_DELAY_1
                     ).pass_through_delay(0, 2, 3, 4)
    # S2: alu = H_a = (thr < m); d1 <- f_a
    dp[2].enable_alu(AluOp.IS_LT, AluInp.PREV_DELAY_4, AluInp.PREV_DELAY_0
                     ).pass_through_delay(2, 3, 4
                     ).enable_delay_from_src(DelayInp.PREV_ALU_OUT, 1)
    # S3: alu = m_a = f_a - H_a
    dp[3].enable_alu(AluOp.SUBTRACT, AluInp.PREV_DELAY_1, AluInp.PREV_ALU_OUT
                     ).pass_through_delay(2, 3, 4)
    # S4: alu = m_a*beta; d0 <- m_a
    dp[4].enable_alu(AluOp.MULTIPLY, AluInp.PREV_ALU_OUT, AluInp.PREV_DELAY_3
                     ).pass_through_delay(2, 4
                     ).enable_delay_from_src(DelayInp.PREV_ALU_OUT, 0)
    # S5: alu = f_b = m_a*beta + c_odd
    dp[5].enable_alu(AluOp.ADD, AluInp.PREV_ALU_OUT, AluInp.PREV_DELAY_2
                     ).pass_through_delay(0, 4)
    # S6: alu = H_b = (thr < m_a); d1 <- f_b
    dp[6].enable_alu(AluOp.IS_LT, AluInp.PREV_DELAY_4, AluInp.PREV_DELAY_0
                     ).pass_through_delay(0
                     ).enable_delay_from_src(DelayInp.PREV_ALU_OUT, 1)
    # S7: alu = m_b = f_b - H_b
    dp[7].enable_alu(AluOp.SUBTRACT, AluInp.PREV_DELAY_1, AluInp.PREV_ALU_OUT
                     ).pass_through_delay(0)
    u.enable_output(OutSel.DELAY_0, OutPath.WR0_LO)   # m_even
    u.enable_output(OutSel.ALU_OUT, OutPath.WR0_HI)   # m_odd
    return u


def _get_dbl_op():
    """Register the pair-interleaved double-step LIF op. The REGULAR (1x)
    variant is the plain single-step program — it is semantically WRONG for
    the pair layout and acts as a loud canary should the engine ever fall
    back (our APs always qualify for 2X_1PORT)."""
    global _DBL_OP
    if _DBL_OP is not None:
        return _DBL_OP
    name = "LIF_DBL_ANT59"
    for op in dve_ops.OPS:
        if op.name == name:
            _DBL_OP = op
            return op

    def _ref(in0, in1, s0, s1, imm2):
        a = in0.astype(np.float32)
        c = np.asarray(in1, np.float32).reshape(a.shape)
        m = a[..., 1::2]
        f_a = m * np.float32(s0) + c[..., 0::2]
        m_a = f_a - (m > s1).astype(np.float32)
        f_b = m_a * np.float32(s0) + c[..., 1::2]
        m_b = f_b - (m_a > s1).astype(np.float32)
        out = np.empty_like(a)
        out[..., 0::2] = m_a
        out[..., 1::2] = m_b
        return out.astype(np.float32)

    spec = Spec(
        body=(Src0 * C0 + Src1) - (Src0 > C1),
        reference=_ref,
    )
    row = dve_ops._CUSTOM_DVE_ROW_BASE + len(dve_ops.OPS)
    assert row < 0x20
    dve_ops._SUB_OPCODE_FOR_NAME[name] = row
    compiled = DveOpSpec(
        name=name, opcode=row, uops=lower(spec, ver="v3"), rd1_en=True,
        uops_2x=[_lif_dbl_2x_uop()], perf_max=1,
    )
    compiled.validate("v3")
    op = dve_ops.DveOp(name, spec, subdim=False,
                       uops_sha={"v3": compiled.sha("v3")})
    dve_ops.OPS.append(op)
    dve_ops.CUSTOM_DVE_SPECS[name] = spec
    dve_ops._COMPILE_CACHE[(name, "v3")] = compiled
    _DBL_OP = op
    return op


def _lif_2x_uop():
    """2X_1PORT uOp program for the LIF step: each cycle the engine reads a
    packed pair of fp16 elements per port (SRC_* = element 0, SRC_*_HI =
    element 1). Element 0's chain runs on ALU stages 0-3, element 1's on
    stages 4-7; results go out packed via WR0_LO/WR0_HI."""
    u = UopConfig()
    u.enable_input(InpSel.SRC_0, 1)      # delay0 = m_e0
    u.enable_input(InpSel.SRC_0_HI, 2)   # delay1 = m_e1
    u.enable_input(InpSel.SRC_1, 3)      # delay2 = c_e0
    u.enable_input(InpSel.SRC_1_HI, 4)   # delay3 = c_e1
    u.enable_input(InpSel.CONST_0, 5)    # delay4 = beta
    u.enable_input(InpSel.CONST_1, 6)    # delay5 = thr
    u.require_inp0 = 1
    u.require_inp1 = 1
    u.trigger = (Trigger.SRC_TENSOR_DONE, Trigger.NONE, Trigger.NONE)
    u.next_uop = (0, 0, 0)
    dp = u.datapath_config
    # S0: alu = m0*beta
    dp[0].enable_alu(AluOp.MULTIPLY, AluInp.PREV_DELAY_0, AluInp.PREV_DELAY_4
                     ).pass_through_delay(0, 1, 2, 3, 4, 5)
    # S1: alu = f0 = m0*beta + c0
    dp[1].enable_alu(AluOp.ADD, AluInp.PREV_ALU_OUT, AluInp.PREV_DELAY_2
                     ).pass_through_delay(0, 1, 3, 4, 5)
    # S2: alu = H0 = (thr < m0); d2 <- f0
    dp[2].enable_alu(AluOp.IS_LT, AluInp.PREV_DELAY_5, AluInp.PREV_DELAY_0
                     ).pass_through_delay(1, 3, 4, 5
                     ).enable_delay_from_src(DelayInp.PREV_ALU_OUT, 2)
    # S3: alu = out0 = f0 - H0
    dp[3].enable_alu(AluOp.SUBTRACT, AluInp.PREV_DELAY_2, AluInp.PREV_ALU_OUT
                     ).pass_through_delay(1, 3, 4, 5)
    # S4: alu = m1*beta; d0 <- out0
    dp[4].enable_alu(AluOp.MULTIPLY, AluInp.PREV_DELAY_1, AluInp.PREV_DELAY_4
                     ).pass_through_delay(1, 3, 5
                     ).enable_delay_from_src(DelayInp.PREV_ALU_OUT, 0)
    # S5: alu = f1 = m1*beta + c1
    dp[5].enable_alu(AluOp.ADD, AluInp.PREV_ALU_OUT, AluInp.PREV_DELAY_3
                     ).pass_through_delay(0, 1, 5)
    # S6: alu = H1 = (thr < m1); d2 <- f1
    dp[6].enable_alu(AluOp.IS_LT, AluInp.PREV_DELAY_5, AluInp.PREV_DELAY_1
                     ).pass_through_delay(0
                     ).enable_delay_from_src(DelayInp.PREV_ALU_OUT, 2)
    # S7: alu = out1 = f1 - H1
    dp[7].enable_alu(AluOp.SUBTRACT, AluInp.PREV_DELAY_2, AluInp.PREV_ALU_OUT
                     ).pass_through_delay(0)
    u.enable_output(OutSel.DELAY_0, OutPath.WR0_LO)   # out0
    u.enable_output(OutSel.ALU_OUT, OutPath.WR0_HI)   # out1
    return u


def _get_lif_op():
    """Register the fused LIF-step op in the custom-DVE table (idempotent)."""
    global _LIF_OP
    if _LIF_OP is not None:
        return _LIF_OP
    name = "LIF_STEP_ANT59"
    for op in dve_ops.OPS:
        if op.name == name:
            _LIF_OP = op
            return op
    spec = Spec(
        body=(Src0 * C0 + Src1) - (Src0 > C1),
        reference=lambda in0, in1, s0, s1, imm2: (
            (in0.astype(np.float32) * np.float32(s0)
             + in1.reshape(in0.shape))
            - (in0 > s1).astype(np.float32)
        ).astype(np.float32),
    )
    row = dve_ops._CUSTOM_DVE_ROW_BASE + len(dve_ops.OPS)
    assert row < 0x20
    dve_ops._SUB_OPCODE_FOR_NAME[name] = row
    compiled = DveOpSpec(
        name=name, opcode=row, uops=lower(spec, ver="v3"), rd1_en=True,
        uops_2x=[_lif_2x_uop()] if LIF2X else None,
        perf_max=1 if LIF2X else 0,
    )
    compiled.validate("v3")
    op = dve_ops.DveOp(name, spec, subdim=False,
                       uops_sha={"v3": compiled.sha("v3")})
    dve_ops.OPS.append(op)
    dve_ops.CUSTOM_DVE_SPECS[name] = spec
    dve_ops._COMPILE_CACHE[(name, "v3")] = compiled
    _LIF_OP = op
    return op


def _build_nc(T):
    """Build the per-core Bass program (SPMD: same program on every core).

    Pair-interleaved layout: everything column-ordered by (super s, sample c,
    parity e) with tick = 2s+e. Ring/drive col = s*256 + 2i + e (state index
    i: m1 i=g*32+c, m2 i=96+c); sigma col = g*512 + s*64 + 2c + e; psum bank
    col = s*64 + 2c + e; host orders the conv rhs columns the same way."""
    dbl = _get_dbl_op()
    ticks = T + SKEW                       # DVE ticks 0..T+SKEW-1
    nblk = -(-ticks // BLK)
    pad_ticks = nblk * BLK
    windows = -(-pad_ticks // WIN)
    xt_cols = windows * WIN * BC
    NB = BLK * BC                          # 512 sample-ticks per block
    SB = BLK // 2                          # 8 super-ticks per block
    SP = 256                               # ring cols per super-tick

    nc = bacc.Bacc("TRN2", target_bir_lowering=False)

    KS = 3 * KX                            # stacked conv K: [xh; xl; xh]
    xts_d = nc.dram_tensor("xts", [KS, xt_cols], BF16, kind="ExternalInput")
    wes_d = nc.dram_tensor("wes", [KS, F], BF16, kind="ExternalInput")
    fch_d = nc.dram_tensor("fch", [128, G * J], F16, kind="ExternalInput")
    fcb_d = nc.dram_tensor("fcb", [128, 1], F32, kind="ExternalInput")
    hist_d = nc.dram_tensor("hist", [J, BC * T], F16, kind="ExternalOutput")

    with tile.TileContext(nc) as tc:
        with (
            tc.tile_pool(name="konst", bufs=1) as kp,
            tc.tile_pool(name="ring", bufs=1) as rp,
            tc.tile_pool(name="sig", bufs=3) as sgp,
            tc.tile_pool(name="xwin", bufs=3) as xp,
            tc.tile_pool(name="cdrv", bufs=3) as cbp,
            tc.tile_pool(name="cpsum", bufs=2, space="PSUM") as cp,
        ):
            # constants -> SBUF
            wes = kp.tile([KS, F], BF16, tag="wes")
            fch = kp.tile([128, G * J], F16, tag="fch")
            fcb = kp.tile([128, 1], F32, tag="fcb")
            for sb, dr in ((wes, wes_d), (fch, fch_d), (fcb, fcb_d)):
                nc.sync.dma_start(sb[:], dr[:])

            # state ring: 3 block-sized fp16 tiles of 16 slices each (the
            # third buys WAR slack so late hist DMAs don't stall the DVE)
            ringA = rp.tile([128, BLK * 128], F16, tag="ringA")
            ringB = rp.tile([128, BLK * 128], F16, tag="ringB")
            ringC = rp.tile([128, BLK * 128], F16, tag="ringC")
            ringD = rp.tile([128, BLK * 128], F16, tag="ringD")
            rings = (ringA, ringB, ringC, ringD)
            NR = len(rings)
            # only block 0's bridge reads pre-existing ring state (the last
            # super of rings[-1]); everything else is written before read
            nc.vector.memset(rings[-1][:, (BLK // 2 - 1) * 256:], 0.0)

            xts = {}      # window idx -> xt sbuf tile
            chs = {}      # block idx -> PSUM C tile [128, 4*512] bank-major
            csts = {}     # block idx -> SBUF fp32 drive tile, tick-major

            def load_window(w):
                # chunked into 10 DMAs so latency-critical hist DMAs behind
                # them on the same queues wait ~1us, not the whole window
                if w < 0 or w >= windows or w in xts:
                    return
                ts = xp.tile([KS, WIN * BC], BF16, tag="xws")
                step = WIN * BC // 10
                for i in range(10):
                    # window 0 gates the whole pipeline start and runs before
                    # any hist DMAs exist: split its chunks across both the
                    # sync and pool queues to halve the warmup latency
                    eng = nc.gpsimd if (w == 0 and i % 2 == 1) else nc.sync
                    eng.dma_start(
                        ts[:, i * step:(i + 1) * step],
                        xts_d[:, w * WIN * BC + i * step:w * WIN * BC + (i + 1) * step],
                    )
                xts[w] = ts

            def ensure_psum(b):
                """Allocate block b's bank-major PSUM C tile (banks 0-2 conv,
                bank 3 fc; psum col = g*512 + t*32 + c)."""
                if b >= nblk or b in chs:
                    return
                ch = cp.tile([128, 4 * NB], F32, tag="ch")
                chs[b] = ch
                w = (b * BLK) // WIN
                base = (b * BLK - w * WIN) * BC
                for g in range(G):
                    nc.tensor.matmul(
                        out=ch[:, g * NB:(g + 1) * NB],
                        lhsT=wes[:, g * 128:(g + 1) * 128],
                        rhs=xts[w][:, base:base + NB],
                        start=True, stop=True,
                    )

            def conv_copies(b):
                """ACT: op1 drains the 3 conv psum banks into the
                pair-interleaved fp16 SBUF drive tile; op2 drains the fc
                bank ADDING the per-partition fc bias (so the fc bias needs
                no matmul and no psum priming)."""
                if b >= nblk or b in csts:
                    return
                cs = cbp.tile([128, BLK * 128], F16, tag="cs")
                csts[b] = cs
                # cst col = s*256 + g*64 + x (x = 2c+e), iterated (g, s, x)
                cs4 = cs[:].rearrange(
                    "p (s g x) -> p g s x", s=SB, g=4, x=64)
                if b >= SKEW // BLK:
                    # ONE merged 4-bank drain + per-partition fc bias; the
                    # conv ones-row weights pre-subtract fcb[p] host-side so
                    # the m1 banks come out exact
                    nc.scalar.activation(
                        out=cs4[:, :, :, :],
                        in_=chs[b][:].rearrange(
                            "p (g s x) -> p g s x", g=4, s=SB, x=64),
                        func=AF.Identity, bias=fcb[:],
                    )
                else:
                    nc.scalar.activation(
                        out=cs4[:, 0:G, :, :],
                        in_=chs[b][:, 0:G * NB].rearrange(
                            "p (g s x) -> p g s x", g=G, s=SB, x=64),
                        func=AF.Identity, bias=fcb[:],
                    )
                    nc.vector.memset(cs4[:, G, :, :], 0.0)

            def spikes_and_fc(b):
                """After block b's ticks: sigma = (m1 >= 1) in {0,1} fp16
                (one DVE stock tensor_scalar reading packed (c,e) pairs so
                the fw 2x/4x perf modes stay eligible), then fc (3 fp16
                matmuls, start=True on g0) into C tile b+SKEW/BLK."""
                lead = SKEW // BLK
                if b < 0 or b + lead >= nblk:
                    return
                # (c, e) pairs are contiguous 64-element runs in both ring
                # and sigma layouts — coalesce so the fw 4x perf mode holds;
                # iterate time-major (s, g, x) like the fast pre-pair layout
                ring4s = rings[b % NR][:].rearrange(
                    "p (s g x) -> p s g x", s=SB, g=4, x=64)
                sg = sgp.tile([128, G * NB], F16, tag="sg")
                sg4 = sg[:].rearrange(
                    "p (g s x) -> p s g x", g=G, s=SB, x=64)
                nc.vector.tensor_scalar(
                    out=sg4, in0=ring4s[:, :, 0:G, :],
                    scalar1=THR, scalar2=None, op0=ALU.is_ge,
                )
                for g in range(G):
                    nc.tensor.matmul(
                        out=chs[b + lead][0:J, G * NB:4 * NB],
                        lhsT=fch[:, g * J:(g + 1) * J],
                        rhs=sg[:, g * NB:(g + 1) * NB],
                        start=(g == 0), stop=(g == G - 1),
                        skip_group_check=True,
                    )

            def hist_dma(b):
                """mem2 of DVE-tick block b = m2 ticks [16b-SKEW, ...):
                DMA straight from the fp16 ring to DRAM (host sums).
                hist col stays t*BC + c; ring src is (s, c, e)."""
                t0 = b * BLK - SKEW
                if t0 < 0:
                    return
                n = min(BLK, T - t0)
                if n <= 0:
                    return
                # hist keeps the ring's native pair order: global col =
                # 64*(t0/2 + s) + 2c + e; the host sums over (s, e) anyway
                ring3 = rings[b % NR][:].rearrange(
                    "p (s x) -> p s x", s=SB, x=SP)
                dst3 = hist_d[:, t0 * BC:(t0 + n) * BC].rearrange(
                    "j (s x) -> j s x", s=n // 2, x=64)
                # issue from the (idle) Pool engine so hist DMAs don't queue
                # behind the x-window loads on the sync engine's DMA queues
                nc.gpsimd.dma_start(
                    dst3, ring3[0:J, 0:n // 2, 192:256],
                )

            # prologue: drive pipeline primed one block deep
            load_window(0)
            load_window(1)
            ensure_psum(0)
            ensure_psum(1)
            conv_copies(0)
            conv_copies(1)

            for b in range(nblk):
                load_window((b * BLK) // WIN + 2)
                ensure_psum(b + 2)
                # lead=5: use sigma from THREE iterations ago so the fc
                # matmuls never gate the psum drain, and drain a full iter
                # ahead of the LIF consumer
                spikes_and_fc(b - 3)
                conv_copies(b + 2)
                hist_dma(b - 1)

                ring = rings[b % NR]
                prev = rings[(b - 1) % NR]
                cst = csts[b]
                nt = min(BLK, ticks - b * BLK)      # ticks in this block
                ns = nt // 2                        # super-ticks (nt is even)
                # bridge: super 0 reads the previous tile's last super pair
                nc.vector._custom_dve(
                    dbl,
                    out=ring[:, 0:SP],
                    in0=prev[:, (SB - 1) * SP:SB * SP],
                    in1=cst[:, 0:SP],
                    s0=BETA, s1=THR,
                )
                if ns > 1:
                    # one op for supers 1..ns-1: in0 = own out shifted back
                    # one super; 128 pairs = 128 cycles of write->read
                    # distance covers the self-overlap latency.
                    nc.vector._custom_dve(
                        dbl,
                        out=ring[:, SP:ns * SP],
                        in0=ring[:, 0:(ns - 1) * SP],
                        in1=cst[:, SP:ns * SP],
                        s0=BETA, s1=THR,
                    )
            # epilogue: the last block's mem2 history
            hist_dma(nblk - 1)

    # the Tile scheduling pass re-emits instructions, so the perf-mode
    # request must be stamped on the FINAL instruction list (byte-36 bits
    # 7:6). The 2X_1PORT program IS the double-step semantics — required,
    # not optional (the REGULAR slot is a canary).
    for bb in nc.main_func.blocks:
        for i in bb.instructions:
            if (type(i).__name__ == "InstCustomDveAnt"
                    and i.op_name == dbl.name):
                i.perf_max = 1
    nc.compile()
    return nc


def _bf16_split(a):
    import ml_dtypes
    hi = a.astype(ml_dtypes.bfloat16)
    lo = (a - hi.astype(np.float32)).astype(ml_dtypes.bfloat16)
    return hi, lo


def _host_prep(x, conv_w, conv_b, fc_w, fc_b, T):
    """Build per-core input maps (numpy only)."""
    ticks = T + SKEW
    nblk = -(-ticks // BLK)
    windows = -(-(nblk * BLK) // WIN)
    xt_ticks = windows * WIN

    fcb = np.zeros((128, 1), np.float32)
    fcb[:J, 0] = fc_b

    wexp = np.zeros((KX, F), np.float32)
    for c in range(CH):
        for l in range(LO):
            wexp[l:l + 7, c * LO + l] = conv_w[c, 0, :]
        wexp[L_IN, c * LO:(c + 1) * LO] = conv_b[c]
    # the merged ACT drain adds fcb[p] to EVERY partition; pre-subtract it
    # from the conv ones-row so the m1 drive stays exact (feature f lands on
    # partition f % 128)
    for f in range(F):
        wexp[L_IN, f] -= fcb[f % 128, 0]
    weh, wel = _bf16_split(wexp)
    wes = np.concatenate([weh, weh, wel], axis=0)  # K-stacked [93, F]

    # spikes s = (m1 >= 1) in {0,1}: c2 = fc_w @ s + fc_b with plain fp16
    # weights; the bias rides the ACT fc-bank drain's per-partition bias port
    fcwt = np.zeros((128, G * J), np.float32)
    for g in range(G):
        fcwt[:, g * J:(g + 1) * J] = fc_w[:, g * 128:(g + 1) * 128].T
    fch = fcwt.astype(np.float16)

    in_maps = []
    B = x.shape[0]
    n_cores = B // BC
    for core in range(n_cores):
        xc = x[core * BC:(core + 1) * BC]          # [BC, T, L]
        xt = np.zeros((KX, xt_ticks, BC), np.float32)
        xt[:L_IN, :T, :] = xc.transpose(2, 1, 0)
        xt[L_IN, :T, :] = 1.0
        # pair-interleave: col = s*64 + 2c + e with tick = 2s + e
        xt = (xt.reshape(KX, xt_ticks // 2, 2, BC)
                .transpose(0, 1, 3, 2)
                .reshape(KX, xt_ticks * BC))
        xth, xtl = _bf16_split(xt)
        xstk = np.concatenate([xth, xtl, xth], axis=0)  # [93, cols]
        in_maps.append({
            "xts": xstk, "wes": wes, "fch": fch, "fcb": fcb,
        })
    return in_maps


def _install_trace_hook():
    """Wire up the axon NTFF profiling hook (absent from this image)."""
    import types

    if "antenv.axon_hooks" in sys.modules:
        return True
    try:
        if "/root/.axon_site" not in sys.path:
            sys.path.insert(0, "/root/.axon_site")
        from trn_agent_boot.trn_boot import _ntff_profile_via_ctypes

        hook = _ntff_profile_via_ctypes("/opt/axon/libaxon_pjrt.so")
        if hook is None:
            return False
        mod = types.ModuleType("antenv.axon_hooks")
        mod.get_axon_ntff_profile_hook = lambda: hook
        sys.modules["antenv.axon_hooks"] = mod
        import concourse.bass_utils as bu

        bu.upload_artifacts = lambda tmpdir: str(tmpdir)
        return True
    except Exception as e:  # profiling is optional
        print(f"trace hook install failed: {e}", file=sys.stderr)
        return False


def run_cores(x, conv_w, conv_b, fc_w, fc_b, T=None):
    """Run the Bass kernel on len(batch)/32 cores; returns [B, 35] output."""
    global LAST_RESULTS
    T = T if T is not None else x.shape[1]
    trace = TRACE and _install_trace_hook()
    nc = _build_nc(T)
    in_maps = _host_prep(x, conv_w, conv_b, fc_w, fc_b, T)
    res = run_bass_kernel_spmd(
        nc, in_maps, core_ids=list(range(len(in_maps))), trace=trace,
    )
    LAST_RESULTS = res
    outs = []
    for i in range(len(in_maps)):
        hv = np.asarray(res.results[i]["hist"], dtype=np.float32)
        # pair-interleaved: col = 64*s + 2c + e -> [J, T/2, sample, parity]
        m2 = hv.reshape(J, T // 2, BC, 2)
        outs.append((m2.sum(axis=(1, 3)) / np.float32(T)).T.astype(np.float32))
    return np.concatenate(outs, axis=0)


def kernel(x, conv_w, conv_b, fc_w, fc_b):
    return run_cores(
        np.asarray(x, np.float32), np.asarray(conv_w, np.float32),
        np.asarray(conv_b, np.float32), np.asarray(fc_w, np.float32),
        np.asarray(fc_b, np.float32),
    )


# revision 54
# speedup vs baseline: 1.0313x; 1.0313x over previous
"""Trainium2 Bass kernel for nn_BasicClassifier (spiking conv classifier).

Sharding: pure data parallelism — batch 256 is split 32 samples per core
across 8 NeuronCores; params are replicated (tiny).

Per-core design (~154us). The T=1000 LIF scan is sequential; everything is
column-ordered PAIR-INTERLEAVED by (super-tick s, sample c, parity e) with
tick = 2s+e so the DVE can run a custom 2X_1PORT "double-step" op:

  - State ring: fp16 [128, 2048] tiles (x4). Ring col s*256 + 2i + e holds
    membrane i at tick 2s+e (i<96: layer-1 feature g*32+c; i>=96: layer-2
    unit row x sample col), lagged SKEW=64 ticks for layer 2.
  - LIF_DBL_ANT59: hand-written 2X_1PORT uOp program computing TWO LIF
    steps  m' = (m*0.9 + c) - (m > 1)  per cycle-column: reads the in0
    pair (HI = prev odd-tick state), the in1 drive pair (c_even, c_odd),
    chains both steps through the 8 ALU stages, writes the (m_e, m_o) pair
    via WR0_LO/HI. Recurrence lag = 128 pairs = 128 cycles of write->read
    distance (measured safe; 64 cycles is NOT — see pitch sweep). Two ops
    per 16-tick block: a 1-super bridge + a fused op whose in0 is its own
    out shifted one super back. perf_max=1 must be stamped on the FINAL
    instruction list (the Tile scheduler re-emits instructions).
  - PE: 3 conv matmuls (bf16 [xh;xl;xh]x[wh;wh;wl] K-stacks, exact to
    ~2^-16) + 3 fc matmuls (fp16 weights over sigma=(m1>=1) in {0,1},
    start=True on g0) per block. No bias matmul anywhere.
  - ACT: ONE merged 4-bank psum drain into the fp16 drive tile with the fc
    bias added via the per-partition bias port; the conv ones-row weights
    pre-subtract fcb[f%128] host-side so the m1 banks stay exact.
  - DVE stock tensor_scalar(is_ge) makes sigma (runs the fw 2x/4x modes).
  - fc sigma lead = 2 blocks (SKEW=4*BLK) so fc matmuls never wait on the
    freshest DVE block.
  - mem2 history: DMA from ring cols (s,192:256) via the POOL engine's DMA
    queues (latency-critical ring WAR; keeps them off the x-window queue).
    Host sums over (s, e).
"""

import os
import sys

for _p in ("/opt/trn_rl_repo", "/opt/pypackages"):
    if _p not in sys.path:
        sys.path.insert(0, _p)

import numpy as np

import concourse.bacc as bacc
import concourse.mybir as mybir
import concourse.tile as tile
import concourse.dve_ops as dve_ops
from concourse.dve_spec import Spec, Src0, Src1, C0, C1, lower
from concourse.dve_uop import (
    AluInp,
    AluOp,
    DelayInp,
    DveOpSpec,
    InpSel,
    OutPath,
    OutSel,
    Trigger,
    UopConfig,
)
from concourse.bass_utils import run_bass_kernel_spmd

F32 = mybir.dt.float32
F16 = mybir.dt.float16
BF16 = mybir.dt.bfloat16
ALU = mybir.AluOpType
AF = mybir.ActivationFunctionType

N_CORES = 8
B_FULL, T_FULL, L_IN = 256, 1000, 30
BC = B_FULL // N_CORES      # 32 samples per core
CH, LO = 16, 24
F = CH * LO                 # 384 features
G = 3                       # feature groups of 128
J = 35                      # fc outputs
KX = L_IN + 1               # conv contraction rows (30 taps + ones row)
BLK = 16                    # ticks per block (= 4 PSUM banks of drive)
SKEW = 5 * BLK              # layer-2 lag: spikes at tick t drive m2 at t+SKEW
WIN = 160                   # ticks per x-window DMA (multiple of BLK)
BETA, THR = 0.9, 1.0

TRACE = bool(int(os.environ.get("KERNEL_TRACE", "0")))
LIF2X = bool(int(os.environ.get("KERNEL_LIF2X", "1")))
LAST_RESULTS = None

_LIF_OP = None
_DBL_OP = None


def _lif_dbl_2x_uop():
    """2X_1PORT uOp program computing a DOUBLE LIF step per cycle.

    State is stored pair-interleaved: ring col s*256 + 2i + e holds membrane
    i at tick 2s+e. Each cycle the engine reads one in0 pair (only the HI
    element m = tick 2s-1 state is used), one in1 pair (c_even, c_odd), and
    the 8 ALU stages chain two full LIF steps, emitting the (m_even, m_odd)
    pair via WR0_LO/WR0_HI. The recurrence lag is one super-tick = 128
    pairs = 128 cycles of write->read distance (the same margin the proven
    1x single-step design has)."""
    u = UopConfig()
    u.enable_input(InpSel.SRC_0_HI, 1)   # d0 = m (prev odd-tick state)
    u.enable_input(InpSel.SRC_1, 2)      # d1 = c_even
    u.enable_input(InpSel.SRC_1_HI, 3)   # d2 = c_odd
    u.enable_input(InpSel.CONST_0, 4)    # d3 = beta
    u.enable_input(InpSel.CONST_1, 5)    # d4 = thr
    u.require_inp0 = 1
    u.require_inp1 = 1
    u.trigger = (Trigger.SRC_TENSOR_DONE, Trigger.NONE, Trigger.NONE)
    u.next_uop = (0, 0, 0)
    dp = u.datapath_config
    # S0: alu = m*beta
    dp[0].enable_alu(AluOp.MULTIPLY, AluInp.PREV_DELAY_0, AluInp.PREV_DELAY_3
                     ).pass_through_delay(0, 1, 2, 3, 4)
    # S1: alu = f_a = m*beta + c_even
    dp[1].enable_alu(AluOp.ADD, AluInp.PREV_ALU_OUT, AluInp.PREV_DELAY_1
                     ).pass_through_delay(0, 2, 3, 4)
    # S2: alu = H_a = (thr < m); d1 <- f_a
    dp[2].enable_alu(AluOp.IS_LT, AluInp.PREV_DELAY_4, AluInp.PREV_DELAY_0
                     ).pass_through_delay(2, 3, 4
                     ).enable_delay_from_src(DelayInp.PREV_ALU_OUT, 1)
    # S3: alu = m_a = f_a - H_a
    dp[3].enable_alu(AluOp.SUBTRACT, AluInp.PREV_DELAY_1, AluInp.PREV_ALU_OUT
                     ).pass_through_delay(2, 3, 4)
    # S4: alu = m_a*beta; d0 <- m_a
    dp[4].enable_alu(AluOp.MULTIPLY, AluInp.PREV_ALU_OUT, AluInp.PREV_DELAY_3
                     ).pass_through_delay(2, 4
                     ).enable_delay_from_src(DelayInp.PREV_ALU_OUT, 0)
    # S5: alu = f_b = m_a*beta + c_odd
    dp[5].enable_alu(AluOp.ADD, AluInp.PREV_ALU_OUT, AluInp.PREV_DELAY_2
                     ).pass_through_delay(0, 4)
    # S6: alu = H_b = (thr < m_a); d1 <- f_b
    dp[6].enable_alu(AluOp.IS_LT, AluInp.PREV_DELAY_4, AluInp.PREV_DELAY_0
                     ).pass_through_delay(0
                     ).enable_delay_from_src(DelayInp.PREV_ALU_OUT, 1)
    # S7: alu = m_b = f_b - H_b
    dp[7].enable_alu(AluOp.SUBTRACT, AluInp.PREV_DELAY_1, AluInp.PREV_ALU_OUT
                     ).pass_through_delay(0)
    u.enable_output(OutSel.DELAY_0, OutPath.WR0_LO)   # m_even
    u.enable_output(OutSel.ALU_OUT, OutPath.WR0_HI)   # m_odd
    return u


def _get_dbl_op():
    """Register the pair-interleaved double-step LIF op. The REGULAR (1x)
    variant is the plain single-step program — it is semantically WRONG for
    the pair layout and acts as a loud canary should the engine ever fall
    back (our APs always qualify for 2X_1PORT)."""
    global _DBL_OP
    if _DBL_OP is not None:
        return _DBL_OP
    name = "LIF_DBL_ANT59"
    for op in dve_ops.OPS:
        if op.name == name:
            _DBL_OP = op
            return op

    def _ref(in0, in1, s0, s1, imm2):
        a = in0.astype(np.float32)
        c = np.asarray(in1, np.float32).reshape(a.shape)
        m = a[..., 1::2]
        f_a = m * np.float32(s0) + c[..., 0::2]
        m_a = f_a - (m > s1).astype(np.float32)
        f_b = m_a * np.float32(s0) + c[..., 1::2]
        m_b = f_b - (m_a > s1).astype(np.float32)
        out = np.empty_like(a)
        out[..., 0::2] = m_a
        out[..., 1::2] = m_b
        return out.astype(np.float32)

    spec = Spec(
        body=(Src0 * C0 + Src1) - (Src0 > C1),
        reference=_ref,
    )
    row = dve_ops._CUSTOM_DVE_ROW_BASE + len(dve_ops.OPS)
    assert row < 0x20
    dve_ops._SUB_OPCODE_FOR_NAME[name] = row
    compiled = DveOpSpec(
        name=name, opcode=row, uops=lower(spec, ver="v3"), rd1_en=True,
        uops_2x=[_lif_dbl_2x_uop()], perf_max=1,
    )
    compiled.validate("v3")
    op = dve_ops.DveOp(name, spec, subdim=False,
                       uops_sha={"v3": compiled.sha("v3")})
    dve_ops.OPS.append(op)
    dve_ops.CUSTOM_DVE_SPECS[name] = spec
    dve_ops._COMPILE_CACHE[(name, "v3")] = compiled
    _DBL_OP = op
    return op


def _lif_2x_uop():
    """2X_1PORT uOp program for the LIF step: each cycle the engine reads a
    packed pair of fp16 elements per port (SRC_* = element 0, SRC_*_HI =
    element 1). Element 0's chain runs on ALU stages 0-3, element 1's on
    stages 4-7; results go out packed via WR0_LO/WR0_HI."""
    u = UopConfig()
    u.enable_input(InpSel.SRC_0, 1)      # delay0 = m_e0
    u.enable_input(InpSel.SRC_0_HI, 2)   # delay1 = m_e1
    u.enable_input(InpSel.SRC_1, 3)      # delay2 = c_e0
    u.enable_input(InpSel.SRC_1_HI, 4)   # delay3 = c_e1
    u.enable_input(InpSel.CONST_0, 5)    # delay4 = beta
    u.enable_input(InpSel.CONST_1, 6)    # delay5 = thr
    u.require_inp0 = 1
    u.require_inp1 = 1
    u.trigger = (Trigger.SRC_TENSOR_DONE, Trigger.NONE, Trigger.NONE)
    u.next_uop = (0, 0, 0)
    dp = u.datapath_config
    # S0: alu = m0*beta
    dp[0].enable_alu(AluOp.MULTIPLY, AluInp.PREV_DELAY_0, AluInp.PREV_DELAY_4
                     ).pass_through_delay(0, 1, 2, 3, 4, 5)
    # S1: alu = f0 = m0*beta + c0
    dp[1].enable_alu(AluOp.ADD, AluInp.PREV_ALU_OUT, AluInp.PREV_DELAY_2
                     ).pass_through_delay(0, 1, 3, 4, 5)
    # S2: alu = H0 = (thr < m0); d2 <- f0
    dp[2].enable_alu(AluOp.IS_LT, AluInp.PREV_DELAY_5, AluInp.PREV_DELAY_0
                     ).pass_through_delay(1, 3, 4, 5
                     ).enable_delay_from_src(DelayInp.PREV_ALU_OUT, 2)
    # S3: alu = out0 = f0 - H0
    dp[3].enable_alu(AluOp.SUBTRACT, AluInp.PREV_DELAY_2, AluInp.PREV_ALU_OUT
                     ).pass_through_delay(1, 3, 4, 5)
    # S4: alu = m1*beta; d0 <- out0
    dp[4].enable_alu(AluOp.MULTIPLY, AluInp.PREV_DELAY_1, AluInp.PREV_DELAY_4
                     ).pass_through_delay(1, 3, 5
                     ).enable_delay_from_src(DelayInp.PREV_ALU_OUT, 0)
    # S5: alu = f1 = m1*beta + c1
    dp[5].enable_alu(AluOp.ADD, AluInp.PREV_ALU_OUT, AluInp.PREV_DELAY_3
                     ).pass_through_delay(0, 1, 5)
    # S6: alu = H1 = (thr < m1); d2 <- f1
    dp[6].enable_alu(AluOp.IS_LT, AluInp.PREV_DELAY_5, AluInp.PREV_DELAY_1
                     ).pass_through_delay(0
                     ).enable_delay_from_src(DelayInp.PREV_ALU_OUT, 2)
    # S7: alu = out1 = f1 - H1
    dp[7].enable_alu(AluOp.SUBTRACT, AluInp.PREV_DELAY_2, AluInp.PREV_ALU_OUT
                     ).pass_through_delay(0)
    u.enable_output(OutSel.DELAY_0, OutPath.WR0_LO)   # out0
    u.enable_output(OutSel.ALU_OUT, OutPath.WR0_HI)   # out1
    return u


def _get_lif_op():
    """Register the fused LIF-step op in the custom-DVE table (idempotent)."""
    global _LIF_OP
    if _LIF_OP is not None:
        return _LIF_OP
    name = "LIF_STEP_ANT59"
    for op in dve_ops.OPS:
        if op.name == name:
            _LIF_OP = op
            return op
    spec = Spec(
        body=(Src0 * C0 + Src1) - (Src0 > C1),
        reference=lambda in0, in1, s0, s1, imm2: (
            (in0.astype(np.float32) * np.float32(s0)
             + in1.reshape(in0.shape))
            - (in0 > s1).astype(np.float32)
        ).astype(np.float32),
    )
    row = dve_ops._CUSTOM_DVE_ROW_BASE + len(dve_ops.OPS)
    assert row < 0x20
    dve_ops._SUB_OPCODE_FOR_NAME[name] = row
    compiled = DveOpSpec(
        name=name, opcode=row, uops=lower(spec, ver="v3"), rd1_en=True,
        uops_2x=[_lif_2x_uop()] if LIF2X else None,
        perf_max=1 if LIF2X else 0,
    )
    compiled.validate("v3")
    op = dve_ops.DveOp(name, spec, subdim=False,
                       uops_sha={"v3": compiled.sha("v3")})
    dve_ops.OPS.append(op)
    dve_ops.CUSTOM_DVE_SPECS[name] = spec
    dve_ops._COMPILE_CACHE[(name, "v3")] = compiled
    _LIF_OP = op
    return op


def _build_nc(T):
    """Build the per-core Bass program (SPMD: same program on every core).

    Pair-interleaved layout: everything column-ordered by (super s, sample c,
    parity e) with tick = 2s+e. Ring/drive col = s*256 + 2i + e (state index
    i: m1 i=g*32+c, m2 i=96+c); sigma col = g*512 + s*64 + 2c + e; psum bank
    col = s*64 + 2c + e; host orders the conv rhs columns the same way."""
    dbl = _get_dbl_op()
    ticks = T + SKEW                       # DVE ticks 0..T+SKEW-1
    nblk = -(-ticks // BLK)
    pad_ticks = nblk * BLK
    windows = -(-pad_ticks // WIN)
    xt_cols = windows * WIN * BC
    NB = BLK * BC                          # 512 sample-ticks per block
    SB = BLK // 2                          # 8 super-ticks per block
    SP = 256                               # ring cols per super-tick

    nc = bacc.Bacc("TRN2", target_bir_lowering=False)

    KS = 3 * KX                            # stacked conv K: [xh; xl; xh]
    xts_d = nc.dram_tensor("xts", [KS, xt_cols], BF16, kind="ExternalInput")
    wes_d = nc.dram_tensor("wes", [KS, F], BF16, kind="ExternalInput")
    fch_d = nc.dram_tensor("fch", [128, G * J], F16, kind="ExternalInput")
    fcb_d = nc.dram_tensor("fcb", [128, 1], F32, kind="ExternalInput")
    hist_d = nc.dram_tensor("hist", [J, BC * T], F16, kind="ExternalOutput")

    with tile.TileContext(nc) as tc:
        with (
            tc.tile_pool(name="konst", bufs=1) as kp,
            tc.tile_pool(name="ring", bufs=1) as rp,
            tc.tile_pool(name="sig", bufs=3) as sgp,
            tc.tile_pool(name="xwin", bufs=3) as xp,
            tc.tile_pool(name="cdrv", bufs=3) as cbp,
            tc.tile_pool(name="cpsum", bufs=2, space="PSUM") as cp,
        ):
            # constants -> SBUF
            wes = kp.tile([KS, F], BF16, tag="wes")
            fch = kp.tile([128, G * J], F16, tag="fch")
            fcb = kp.tile([128, 1], F32, tag="fcb")
            for sb, dr in ((wes, wes_d), (fch, fch_d), (fcb, fcb_d)):
                nc.sync.dma_start(sb[:], dr[:])

            # state ring: ONE contiguous fp16 region of 4 block-sized
            # slots so a block whose predecessor slot sits directly before
            # it needs NO bridge op (in0 reads straight across the slot
            # boundary); only every NR-th block (the wrap) bridges
            NR = 4
            RNG = BLK * 128
            bigring = rp.tile([128, NR * RNG], F16, tag="bigring")
            # only block 0's wrap-bridge reads pre-existing ring state (the
            # last super of the region); everything else is written first
            nc.vector.memset(bigring[:, NR * RNG - SP:], 0.0)

            def ring_of(b):
                return bigring[:, (b % NR) * RNG:((b % NR) + 1) * RNG]

            xts = {}      # window idx -> xt sbuf tile
            chs = {}      # block idx -> PSUM C tile [128, 4*512] bank-major
            csts = {}     # block idx -> SBUF fp32 drive tile, tick-major

            def load_window(w):
                # chunked into 10 DMAs so latency-critical hist DMAs behind
                # them on the same queues wait ~1us, not the whole window
                if w < 0 or w >= windows or w in xts:
                    return
                ts = xp.tile([KS, WIN * BC], BF16, tag="xws")
                step = WIN * BC // 10
                for i in range(10):
                    nc.sync.dma_start(
                        ts[:, i * step:(i + 1) * step],
                        xts_d[:, w * WIN * BC + i * step:w * WIN * BC + (i + 1) * step],
                    )
                xts[w] = ts

            def ensure_psum(b):
                """Allocate block b's bank-major PSUM C tile (banks 0-2 conv,
                bank 3 fc; psum col = g*512 + t*32 + c)."""
                if b >= nblk or b in chs:
                    return
                ch = cp.tile([128, 4 * NB], F32, tag="ch")
                chs[b] = ch
                w = (b * BLK) // WIN
                base = (b * BLK - w * WIN) * BC
                for g in range(G):
                    nc.tensor.matmul(
                        out=ch[:, g * NB:(g + 1) * NB],
                        lhsT=wes[:, g * 128:(g + 1) * 128],
                        rhs=xts[w][:, base:base + NB],
                        start=True, stop=True,
                    )

            def conv_copies(b):
                """ACT: op1 drains the 3 conv psum banks into the
                pair-interleaved fp16 SBUF drive tile; op2 drains the fc
                bank ADDING the per-partition fc bias (so the fc bias needs
                no matmul and no psum priming)."""
                if b >= nblk or b in csts:
                    return
                cs = cbp.tile([128, BLK * 128], F16, tag="cs")
                csts[b] = cs
                # cst col = s*256 + g*64 + x (x = 2c+e), iterated (g, s, x)
                cs4 = cs[:].rearrange(
                    "p (s g x) -> p g s x", s=SB, g=4, x=64)
                if b >= SKEW // BLK:
                    # ONE merged 4-bank drain + per-partition fc bias; the
                    # conv ones-row weights pre-subtract fcb[p] host-side so
                    # the m1 banks come out exact
                    nc.scalar.activation(
                        out=cs4[:, :, :, :],
                        in_=chs[b][:].rearrange(
                            "p (g s x) -> p g s x", g=4, s=SB, x=64),
                        func=AF.Identity, bias=fcb[:],
                    )
                else:
                    nc.scalar.activation(
                        out=cs4[:, 0:G, :, :],
                        in_=chs[b][:, 0:G * NB].rearrange(
                            "p (g s x) -> p g s x", g=G, s=SB, x=64),
                        func=AF.Identity, bias=fcb[:],
                    )
                    nc.vector.memset(cs4[:, G, :, :], 0.0)

            def spikes_and_fc(b):
                """After block b's ticks: sigma = (m1 >= 1) in {0,1} fp16
                (one DVE stock tensor_scalar reading packed (c,e) pairs so
                the fw 2x/4x perf modes stay eligible), then fc (3 fp16
                matmuls, start=True on g0) into C tile b+SKEW/BLK."""
                lead = SKEW // BLK
                if b < 0 or b + lead >= nblk:
                    return
                # (c, e) pairs are contiguous 64-element runs in both ring
                # and sigma layouts — coalesce so the fw 4x perf mode holds;
                # iterate time-major (s, g, x) like the fast pre-pair layout
                ring4s = ring_of(b).rearrange(
                    "p (s g x) -> p s g x", s=SB, g=4, x=64)
                sg = sgp.tile([128, G * NB], F16, tag="sg")
                sg4 = sg[:].rearrange(
                    "p (g s x) -> p s g x", g=G, s=SB, x=64)
                nc.vector.tensor_scalar(
                    out=sg4, in0=ring4s[:, :, 0:G, :],
                    scalar1=THR, scalar2=None, op0=ALU.is_ge,
                )
                for g in range(G):
                    nc.tensor.matmul(
                        out=chs[b + lead][0:J, G * NB:4 * NB],
                        lhsT=fch[:, g * J:(g + 1) * J],
                        rhs=sg[:, g * NB:(g + 1) * NB],
                        start=(g == 0), stop=(g == G - 1),
                        skip_group_check=True,
                    )

            def hist_dma(b):
                """mem2 of DVE-tick block b = m2 ticks [16b-SKEW, ...):
                DMA straight from the fp16 ring to DRAM (host sums).
                hist col stays t*BC + c; ring src is (s, c, e)."""
                t0 = b * BLK - SKEW
                if t0 < 0:
                    return
                n = min(BLK, T - t0)
                if n <= 0:
                    return
                # hist keeps the ring's native pair order: global col =
                # 64*(t0/2 + s) + 2c + e; the host sums over (s, e) anyway
                ring3 = ring_of(b).rearrange(
                    "p (s x) -> p s x", s=SB, x=SP)
                dst3 = hist_d[:, t0 * BC:(t0 + n) * BC].rearrange(
                    "j (s x) -> j s x", s=n // 2, x=64)
                # issue from the (idle) Pool engine so hist DMAs don't queue
                # behind the x-window loads on the sync engine's DMA queues
                nc.gpsimd.dma_start(
                    dst3, ring3[0:J, 0:n // 2, 192:256],
                )

            # prologue: drive pipeline primed one block deep
            load_window(0)
            load_window(1)
            ensure_psum(0)
            ensure_psum(1)
            conv_copies(0)
            conv_copies(1)

            for b in range(nblk):
                load_window((b * BLK) // WIN + 2)
                ensure_psum(b + 2)
                # lead=5: use sigma from THREE iterations ago so the fc
                # matmuls never gate the psum drain, and drain a full iter
                # ahead of the LIF consumer
                spikes_and_fc(b - 3)
                conv_copies(b + 2)
                hist_dma(b - 1)

                base = (b % NR) * RNG
                cst = csts[b]
                nt = min(BLK, ticks - b * BLK)      # ticks in this block
                ns = nt // 2                        # super-ticks (nt is even)
                if b % NR == 0:
                    # wrap: super 0's predecessor is the END of the region
                    nc.vector._custom_dve(
                        dbl,
                        out=bigring[:, base:base + SP],
                        in0=bigring[:, NR * RNG - SP:NR * RNG],
                        in1=cst[:, 0:SP],
                        s0=BETA, s1=THR,
                    )
                    if ns > 1:
                        nc.vector._custom_dve(
                            dbl,
                            out=bigring[:, base + SP:base + ns * SP],
                            in0=bigring[:, base:base + (ns - 1) * SP],
                            in1=cst[:, SP:ns * SP],
                            s0=BETA, s1=THR,
                        )
                else:
                    # predecessor slot is contiguous below: ONE op covers
                    # all supers; in0 = own out shifted one super back,
                    # crossing the slot boundary (128-pair RAW distance
                    # within the op; the interleaved sign op spaces the
                    # cross-instruction boundary)
                    nc.vector._custom_dve(
                        dbl,
                        out=bigring[:, base:base + ns * SP],
                        in0=bigring[:, base - SP:base + (ns - 1) * SP],
                        in1=cst[:, 0:ns * SP],
                        s0=BETA, s1=THR,
                    )
            # epilogue: the last block's mem2 history
            hist_dma(nblk - 1)

    # the Tile scheduling pass re-emits instructions, so the perf-mode
    # request must be stamped on the FINAL instruction list (byte-36 bits
    # 7:6). The 2X_1PORT program IS the double-step semantics — required,
    # not optional (the REGULAR slot is a canary).
    for bb in nc.main_func.blocks:
        for i in bb.instructions:
            if (type(i).__name__ == "InstCustomDveAnt"
                    and i.op_name == dbl.name):
                i.perf_max = 1
    nc.compile()
    return nc


def _bf16_split(a):
    import ml_dtypes
    hi = a.astype(ml_dtypes.bfloat16)
    lo = (a - hi.astype(np.float32)).astype(ml_dtypes.bfloat16)
    return hi, lo


def _host_prep(x, conv_w, conv_b, fc_w, fc_b, T):
    """Build per-core input maps (numpy only)."""
    ticks = T + SKEW
    nblk = -(-ticks // BLK)
    windows = -(-(nblk * BLK) // WIN)
    xt_ticks = windows * WIN

    fcb = np.zeros((128, 1), np.float32)
    fcb[:J, 0] = fc_b

    wexp = np.zeros((KX, F), np.float32)
    for c in range(CH):
        for l in range(LO):
            wexp[l:l + 7, c * LO + l] = conv_w[c, 0, :]
        wexp[L_IN, c * LO:(c + 1) * LO] = conv_b[c]
    # the merged ACT drain adds fcb[p] to EVERY partition; pre-subtract it
    # from the conv ones-row so the m1 drive stays exact (feature f lands on
    # partition f % 128)
    for f in range(F):
        wexp[L_IN, f] -= fcb[f % 128, 0]
    weh, wel = _bf16_split(wexp)
    wes = np.concatenate([weh, weh, wel], axis=0)  # K-stacked [93, F]

    # spikes s = (m1 >= 1) in {0,1}: c2 = fc_w @ s + fc_b with plain fp16
    # weights; the bias rides the ACT fc-bank drain's per-partition bias port
    fcwt = np.zeros((128, G * J), np.float32)
    for g in range(G):
        fcwt[:, g * J:(g + 1) * J] = fc_w[:, g * 128:(g + 1) * 128].T
    fch = fcwt.astype(np.float16)

    in_maps = []
    B = x.shape[0]
    n_cores = B // BC
    for core in range(n_cores):
        xc = x[core * BC:(core + 1) * BC]          # [BC, T, L]
        xt = np.zeros((KX, xt_ticks, BC), np.float32)
        xt[:L_IN, :T, :] = xc.transpose(2, 1, 0)
        xt[L_IN, :T, :] = 1.0
        # pair-interleave: col = s*64 + 2c + e with tick = 2s + e
        xt = (xt.reshape(KX, xt_ticks // 2, 2, BC)
                .transpose(0, 1, 3, 2)
                .reshape(KX, xt_ticks * BC))
        xth, xtl = _bf16_split(xt)
        xstk = np.concatenate([xth, xtl, xth], axis=0)  # [93, cols]
        in_maps.append({
            "xts": xstk, "wes": wes, "fch": fch, "fcb": fcb,
        })
    return in_maps


def _install_trace_hook():
    """Wire up the axon NTFF profiling hook (absent from this image)."""
    import types

    if "antenv.axon_hooks" in sys.modules:
        return True
    try:
        if "/root/.axon_site" not in sys.path:
            sys.path.insert(0, "/root/.axon_site")
        from trn_agent_boot.trn_boot import _ntff_profile_via_ctypes

        hook = _ntff_profile_via_ctypes("/opt/axon/libaxon_pjrt.so")
        if hook is None:
            return False
        mod = types.ModuleType("antenv.axon_hooks")
        mod.get_axon_ntff_profile_hook = lambda: hook
        sys.modules["antenv.axon_hooks"] = mod
        import concourse.bass_utils as bu

        bu.upload_artifacts = lambda tmpdir: str(tmpdir)
        return True
    except Exception as e:  # profiling is optional
        print(f"trace hook install failed: {e}", file=sys.stderr)
        return False


def run_cores(x, conv_w, conv_b, fc_w, fc_b, T=None):
    """Run the Bass kernel on len(batch)/32 cores; returns [B, 35] output."""
    global LAST_RESULTS
    T = T if T is not None else x.shape[1]
    trace = TRACE and _install_trace_hook()
    nc = _build_nc(T)
    in_maps = _host_prep(x, conv_w, conv_b, fc_w, fc_b, T)
    res = run_bass_kernel_spmd(
        nc, in_maps, core_ids=list(range(len(in_maps))), trace=trace,
    )
    LAST_RESULTS = res
    outs = []
    for i in range(len(in_maps)):
        hv = np.asarray(res.results[i]["hist"], dtype=np.float32)
        # pair-interleaved: col = 64*s + 2c + e -> [J, T/2, sample, parity]
        m2 = hv.reshape(J, T // 2, BC, 2)
        outs.append((m2.sum(axis=(1, 3)) / np.float32(T)).T.astype(np.float32))
    return np.concatenate(outs, axis=0)


def kernel(x, conv_w, conv_b, fc_w, fc_b):
    return run_cores(
        np.asarray(x, np.float32), np.asarray(conv_w, np.float32),
        np.asarray(conv_b, np.float32), np.asarray(fc_w, np.float32),
        np.asarray(fc_b, np.float32),
    )


# revision 55
# speedup vs baseline: 1.0433x; 1.0116x over previous
"""Trainium2 Bass kernel for nn_BasicClassifier (spiking conv classifier).

Sharding: pure data parallelism — batch 256 is split 32 samples per core
across 8 NeuronCores; params are replicated (tiny).

Per-core design (~154us). The T=1000 LIF scan is sequential; everything is
column-ordered PAIR-INTERLEAVED by (super-tick s, sample c, parity e) with
tick = 2s+e so the DVE can run a custom 2X_1PORT "double-step" op:

  - State ring: fp16 [128, 4*2048] contiguous region of 4 block slots.
    Within a slot, col s*256 + 2i + e holds
    membrane i at tick 2s+e (i<96: layer-1 feature g*32+c; i>=96: layer-2
    unit row x sample col), lagged SKEW=80 ticks for layer 2.
  - LIF_DBL_ANT59: hand-written 2X_1PORT uOp program computing TWO LIF
    steps  m' = (m*0.9 + c) - (m > 1)  per cycle-column: reads the in0
    pair (HI = prev odd-tick state), the in1 drive pair (c_even, c_odd),
    chains both steps through the 8 ALU stages, writes the (m_e, m_o) pair
    via WR0_LO/HI. Recurrence lag = 128 pairs = 128 cycles of write->read
    distance (measured safe; 64 cycles is NOT — see pitch sweep). The 4
    ring slots live in ONE contiguous SBUF region, so a block is ONE fused
    op whose in0 reads across the slot boundary; only every 4th block (the
    wrap) needs a bridge. perf_max=1 must be stamped on the FINAL
    instruction list (the Tile scheduler re-emits instructions).
  - PE: 3 conv matmuls (bf16 [xh;xl;xh]x[wh;wh;wl] K-stacks, exact to
    ~2^-16) + 3 fc matmuls (fp16 weights over sigma=(m1>=1) in {0,1},
    start=True on g0) per block. No bias matmul anywhere.
  - ACT: ONE merged 4-bank psum drain into the fp16 drive tile with the fc
    bias added via the per-partition bias port; the conv ones-row weights
    pre-subtract fcb[f%128] host-side so the m1 banks stay exact.
  - DVE stock tensor_scalar(is_ge) makes sigma (runs the fw 2x/4x modes).
  - fc sigma lead = 3 blocks (SKEW=5*BLK) so the fc matmuls never gate
    the psum drain, which runs a full iteration ahead of its consumer.
  - mem2 history: DMA from ring cols (s,192:256) via the POOL engine's DMA
    queues (latency-critical ring WAR; keeps them off the x-window queue).
    Host sums over (s, e).
"""

import os
import sys

for _p in ("/opt/trn_rl_repo", "/opt/pypackages"):
    if _p not in sys.path:
        sys.path.insert(0, _p)

import numpy as np

import concourse.bacc as bacc
import concourse.mybir as mybir
import concourse.tile as tile
import concourse.dve_ops as dve_ops
from concourse.dve_spec import Spec, Src0, Src1, C0, C1, lower
from concourse.dve_uop import (
    AluInp,
    AluOp,
    DelayInp,
    DveOpSpec,
    InpSel,
    OutPath,
    OutSel,
    Trigger,
    UopConfig,
)
from concourse.bass_utils import run_bass_kernel_spmd

F32 = mybir.dt.float32
F16 = mybir.dt.float16
BF16 = mybir.dt.bfloat16
ALU = mybir.AluOpType
AF = mybir.ActivationFunctionType

N_CORES = 8
B_FULL, T_FULL, L_IN = 256, 1000, 30
BC = B_FULL // N_CORES      # 32 samples per core
CH, LO = 16, 24
F = CH * LO                 # 384 features
G = 3                       # feature groups of 128
J = 35                      # fc outputs
KX = L_IN + 1               # conv contraction rows (30 taps + ones row)
BLK = 16                    # ticks per block (= 4 PSUM banks of drive)
SKEW = 5 * BLK              # layer-2 lag: spikes at tick t drive m2 at t+SKEW
WIN = 160                   # ticks per x-window DMA (multiple of BLK)
BETA, THR = 0.9, 1.0

TRACE = bool(int(os.environ.get("KERNEL_TRACE", "0")))
LIF2X = bool(int(os.environ.get("KERNEL_LIF2X", "1")))
LAST_RESULTS = None

_LIF_OP = None
_DBL_OP = None


def _lif_dbl_2x_uop():
    """2X_1PORT uOp program computing a DOUBLE LIF step per cycle.

    State is stored pair-interleaved: ring col s*256 + 2i + e holds membrane
    i at tick 2s+e. Each cycle the engine reads one in0 pair (only the HI
    element m = tick 2s-1 state is used), one in1 pair (c_even, c_odd), and
    the 8 ALU stages chain two full LIF steps, emitting the (m_even, m_odd)
    pair via WR0_LO/WR0_HI. The recurrence lag is one super-tick = 128
    pairs = 128 cycles of write->read distance (the same margin the proven
    1x single-step design has)."""
    u = UopConfig()
    u.enable_input(InpSel.SRC_0_HI, 1)   # d0 = m (prev odd-tick state)
    u.enable_input(InpSel.SRC_1, 2)      # d1 = c_even
    u.enable_input(InpSel.SRC_1_HI, 3)   # d2 = c_odd
    u.enable_input(InpSel.CONST_0, 4)    # d3 = beta
    u.enable_input(InpSel.CONST_1, 5)    # d4 = thr
    u.require_inp0 = 1
    u.require_inp1 = 1
    u.trigger = (Trigger.SRC_TENSOR_DONE, Trigger.NONE, Trigger.NONE)
    u.next_uop = (0, 0, 0)
    dp = u.datapath_config
    # S0: alu = m*beta
    dp[0].enable_alu(AluOp.MULTIPLY, AluInp.PREV_DELAY_0, AluInp.PREV_DELAY_3
                     ).pass_through_delay(0, 1, 2, 3, 4)
    # S1: alu = f_a = m*beta + c_even
    dp[1].enable_alu(AluOp.ADD, AluInp.PREV_ALU_OUT, AluInp.PREV_DELAY_1
                     ).pass_through_delay(0, 2, 3, 4)
    # S2: alu = H_a = (thr < m); d1 <- f_a
    dp[2].enable_alu(AluOp.IS_LT, AluInp.PREV_DELAY_4, AluInp.PREV_DELAY_0
                     ).pass_through_delay(2, 3, 4
                     ).enable_delay_from_src(DelayInp.PREV_ALU_OUT, 1)
    # S3: alu = m_a = f_a - H_a
    dp[3].enable_alu(AluOp.SUBTRACT, AluInp.PREV_DELAY_1, AluInp.PREV_ALU_OUT
                     ).pass_through_delay(2, 3, 4)
    # S4: alu = m_a*beta; d0 <- m_a
    dp[4].enable_alu(AluOp.MULTIPLY, AluInp.PREV_ALU_OUT, AluInp.PREV_DELAY_3
                     ).pass_through_delay(2, 4
                     ).enable_delay_from_src(DelayInp.PREV_ALU_OUT, 0)
    # S5: alu = f_b = m_a*beta + c_odd
    dp[5].enable_alu(AluOp.ADD, AluInp.PREV_ALU_OUT, AluInp.PREV_DELAY_2
                     ).pass_through_delay(0, 4)
    # S6: alu = H_b = (thr < m_a); d1 <- f_b
    dp[6].enable_alu(AluOp.IS_LT, AluInp.PREV_DELAY_4, AluInp.PREV_DELAY_0
                     ).pass_through_delay(0
                     ).enable_delay_from_src(DelayInp.PREV_ALU_OUT, 1)
    # S7: alu = m_b = f_b - H_b
    dp[7].enable_alu(AluOp.SUBTRACT, AluInp.PREV_DELAY_1, AluInp.PREV_ALU_OUT
                     ).pass_through_delay(0)
    u.enable_output(OutSel.DELAY_0, OutPath.WR0_LO)   # m_even
    u.enable_output(OutSel.ALU_OUT, OutPath.WR0_HI)   # m_odd
    return u


def _get_dbl_op():
    """Register the pair-interleaved double-step LIF op. The REGULAR (1x)
    variant is the plain single-step program — it is semantically WRONG for
    the pair layout and acts as a loud canary should the engine ever fall
    back (our APs always qualify for 2X_1PORT)."""
    global _DBL_OP
    if _DBL_OP is not None:
        return _DBL_OP
    name = "LIF_DBL_ANT59"
    for op in dve_ops.OPS:
        if op.name == name:
            _DBL_OP = op
            return op

    def _ref(in0, in1, s0, s1, imm2):
        a = in0.astype(np.float32)
        c = np.asarray(in1, np.float32).reshape(a.shape)
        m = a[..., 1::2]
        f_a = m * np.float32(s0) + c[..., 0::2]
        m_a = f_a - (m > s1).astype(np.float32)
        f_b = m_a * np.float32(s0) + c[..., 1::2]
        m_b = f_b - (m_a > s1).astype(np.float32)
        out = np.empty_like(a)
        out[..., 0::2] = m_a
        out[..., 1::2] = m_b
        return out.astype(np.float32)

    spec = Spec(
        body=(Src0 * C0 + Src1) - (Src0 > C1),
        reference=_ref,
    )
    row = dve_ops._CUSTOM_DVE_ROW_BASE + len(dve_ops.OPS)
    assert row < 0x20
    dve_ops._SUB_OPCODE_FOR_NAME[name] = row
    compiled = DveOpSpec(
        name=name, opcode=row, uops=lower(spec, ver="v3"), rd1_en=True,
        uops_2x=[_lif_dbl_2x_uop()], perf_max=1,
    )
    compiled.validate("v3")
    op = dve_ops.DveOp(name, spec, subdim=False,
                       uops_sha={"v3": compiled.sha("v3")})
    dve_ops.OPS.append(op)
    dve_ops.CUSTOM_DVE_SPECS[name] = spec
    dve_ops._COMPILE_CACHE[(name, "v3")] = compiled
    _DBL_OP = op
    return op


def _lif_2x_uop():
    """2X_1PORT uOp program for the LIF step: each cycle the engine reads a
    packed pair of fp16 elements per port (SRC_* = element 0, SRC_*_HI =
    element 1). Element 0's chain runs on ALU stages 0-3, element 1's on
    stages 4-7; results go out packed via WR0_LO/WR0_HI."""
    u = UopConfig()
    u.enable_input(InpSel.SRC_0, 1)      # delay0 = m_e0
    u.enable_input(InpSel.SRC_0_HI, 2)   # delay1 = m_e1
    u.enable_input(InpSel.SRC_1, 3)      # delay2 = c_e0
    u.enable_input(InpSel.SRC_1_HI, 4)   # delay3 = c_e1
    u.enable_input(InpSel.CONST_0, 5)    # delay4 = beta
    u.enable_input(InpSel.CONST_1, 6)    # delay5 = thr
    u.require_inp0 = 1
    u.require_inp1 = 1
    u.trigger = (Trigger.SRC_TENSOR_DONE, Trigger.NONE, Trigger.NONE)
    u.next_uop = (0, 0, 0)
    dp = u.datapath_config
    # S0: alu = m0*beta
    dp[0].enable_alu(AluOp.MULTIPLY, AluInp.PREV_DELAY_0, AluInp.PREV_DELAY_4
                     ).pass_through_delay(0, 1, 2, 3, 4, 5)
    # S1: alu = f0 = m0*beta + c0
    dp[1].enable_alu(AluOp.ADD, AluInp.PREV_ALU_OUT, AluInp.PREV_DELAY_2
                     ).pass_through_delay(0, 1, 3, 4, 5)
    # S2: alu = H0 = (thr < m0); d2 <- f0
    dp[2].enable_alu(AluOp.IS_LT, AluInp.PREV_DELAY_5, AluInp.PREV_DELAY_0
                     ).pass_through_delay(1, 3, 4, 5
                     ).enable_delay_from_src(DelayInp.PREV_ALU_OUT, 2)
    # S3: alu = out0 = f0 - H0
    dp[3].enable_alu(AluOp.SUBTRACT, AluInp.PREV_DELAY_2, AluInp.PREV_ALU_OUT
                     ).pass_through_delay(1, 3, 4, 5)
    # S4: alu = m1*beta; d0 <- out0
    dp[4].enable_alu(AluOp.MULTIPLY, AluInp.PREV_DELAY_1, AluInp.PREV_DELAY_4
                     ).pass_through_delay(1, 3, 5
                     ).enable_delay_from_src(DelayInp.PREV_ALU_OUT, 0)
    # S5: alu = f1 = m1*beta + c1
    dp[5].enable_alu(AluOp.ADD, AluInp.PREV_ALU_OUT, AluInp.PREV_DELAY_3
                     ).pass_through_delay(0, 1, 5)
    # S6: alu = H1 = (thr < m1); d2 <- f1
    dp[6].enable_alu(AluOp.IS_LT, AluInp.PREV_DELAY_5, AluInp.PREV_DELAY_1
                     ).pass_through_delay(0
                     ).enable_delay_from_src(DelayInp.PREV_ALU_OUT, 2)
    # S7: alu = out1 = f1 - H1
    dp[7].enable_alu(AluOp.SUBTRACT, AluInp.PREV_DELAY_2, AluInp.PREV_ALU_OUT
                     ).pass_through_delay(0)
    u.enable_output(OutSel.DELAY_0, OutPath.WR0_LO)   # out0
    u.enable_output(OutSel.ALU_OUT, OutPath.WR0_HI)   # out1
    return u


def _get_lif_op():
    """Register the fused LIF-step op in the custom-DVE table (idempotent)."""
    global _LIF_OP
    if _LIF_OP is not None:
        return _LIF_OP
    name = "LIF_STEP_ANT59"
    for op in dve_ops.OPS:
        if op.name == name:
            _LIF_OP = op
            return op
    spec = Spec(
        body=(Src0 * C0 + Src1) - (Src0 > C1),
        reference=lambda in0, in1, s0, s1, imm2: (
            (in0.astype(np.float32) * np.float32(s0)
             + in1.reshape(in0.shape))
            - (in0 > s1).astype(np.float32)
        ).astype(np.float32),
    )
    row = dve_ops._CUSTOM_DVE_ROW_BASE + len(dve_ops.OPS)
    assert row < 0x20
    dve_ops._SUB_OPCODE_FOR_NAME[name] = row
    compiled = DveOpSpec(
        name=name, opcode=row, uops=lower(spec, ver="v3"), rd1_en=True,
        uops_2x=[_lif_2x_uop()] if LIF2X else None,
        perf_max=1 if LIF2X else 0,
    )
    compiled.validate("v3")
    op = dve_ops.DveOp(name, spec, subdim=False,
                       uops_sha={"v3": compiled.sha("v3")})
    dve_ops.OPS.append(op)
    dve_ops.CUSTOM_DVE_SPECS[name] = spec
    dve_ops._COMPILE_CACHE[(name, "v3")] = compiled
    _LIF_OP = op
    return op


def _build_nc(T):
    """Build the per-core Bass program (SPMD: same program on every core).

    Pair-interleaved layout: everything column-ordered by (super s, sample c,
    parity e) with tick = 2s+e. Ring/drive col = s*256 + 2i + e (state index
    i: m1 i=g*32+c, m2 i=96+c); sigma col = g*512 + s*64 + 2c + e; psum bank
    col = s*64 + 2c + e; host orders the conv rhs columns the same way."""
    dbl = _get_dbl_op()
    ticks = T + SKEW                       # DVE ticks 0..T+SKEW-1
    nblk = -(-ticks // BLK)
    pad_ticks = nblk * BLK
    windows = -(-pad_ticks // WIN)
    xt_cols = windows * WIN * BC
    NB = BLK * BC                          # 512 sample-ticks per block
    SB = BLK // 2                          # 8 super-ticks per block
    SP = 256                               # ring cols per super-tick

    nc = bacc.Bacc("TRN2", target_bir_lowering=False)

    KS = 3 * KX                            # stacked conv K: [xh; xl; xh]
    xts_d = nc.dram_tensor("xts", [KS, xt_cols], BF16, kind="ExternalInput")
    wes_d = nc.dram_tensor("wes", [KS, F], BF16, kind="ExternalInput")
    fch_d = nc.dram_tensor("fch", [128, G * J], F16, kind="ExternalInput")
    fcb_d = nc.dram_tensor("fcb", [128, 1], F32, kind="ExternalInput")
    hist_d = nc.dram_tensor("hist", [J, BC * T], F16, kind="ExternalOutput")

    with tile.TileContext(nc) as tc:
        with (
            tc.tile_pool(name="konst", bufs=1) as kp,
            tc.tile_pool(name="ring", bufs=1) as rp,
            tc.tile_pool(name="sig", bufs=3) as sgp,
            tc.tile_pool(name="xwin", bufs=3) as xp,
            tc.tile_pool(name="cdrv", bufs=3) as cbp,
            tc.tile_pool(name="cpsum", bufs=2, space="PSUM") as cp,
        ):
            # constants -> SBUF
            wes = kp.tile([KS, F], BF16, tag="wes")
            fch = kp.tile([128, G * J], F16, tag="fch")
            fcb = kp.tile([128, 1], F32, tag="fcb")
            for sb, dr in ((wes, wes_d), (fch, fch_d), (fcb, fcb_d)):
                nc.sync.dma_start(sb[:], dr[:])

            # state ring: ONE contiguous fp16 region of 4 block-sized
            # slots so a block whose predecessor slot sits directly before
            # it needs NO bridge op (in0 reads straight across the slot
            # boundary); only every NR-th block (the wrap) bridges
            NR = 4
            RNG = BLK * 128
            bigring = rp.tile([128, NR * RNG], F16, tag="bigring")
            # only block 0's wrap-bridge reads pre-existing ring state (the
            # last super of the region); everything else is written first
            nc.vector.memset(bigring[:, NR * RNG - SP:], 0.0)

            def ring_of(b):
                return bigring[:, (b % NR) * RNG:((b % NR) + 1) * RNG]

            xts = {}      # window idx -> xt sbuf tile
            chs = {}      # block idx -> PSUM C tile [128, 4*512] bank-major
            csts = {}     # block idx -> SBUF fp32 drive tile, tick-major

            def load_window(w):
                # chunked into 10 DMAs so latency-critical hist DMAs behind
                # them on the same queues wait ~1us, not the whole window
                if w < 0 or w >= windows or w in xts:
                    return
                ts = xp.tile([KS, WIN * BC], BF16, tag="xws")
                step = WIN * BC // 10
                for i in range(10):
                    nc.sync.dma_start(
                        ts[:, i * step:(i + 1) * step],
                        xts_d[:, w * WIN * BC + i * step:w * WIN * BC + (i + 1) * step],
                    )
                xts[w] = ts

            def ensure_psum(b):
                """Allocate block b's bank-major PSUM C tile (banks 0-2 conv,
                bank 3 fc; psum col = g*512 + t*32 + c)."""
                if b >= nblk or b in chs:
                    return
                ch = cp.tile([128, 4 * NB], F32, tag="ch")
                chs[b] = ch
                w = (b * BLK) // WIN
                base = (b * BLK - w * WIN) * BC
                for g in range(G):
                    nc.tensor.matmul(
                        out=ch[:, g * NB:(g + 1) * NB],
                        lhsT=wes[:, g * 128:(g + 1) * 128],
                        rhs=xts[w][:, base:base + NB],
                        start=True, stop=True,
                    )

            def conv_copies(b):
                """ACT: op1 drains the 3 conv psum banks into the
                pair-interleaved fp16 SBUF drive tile; op2 drains the fc
                bank ADDING the per-partition fc bias (so the fc bias needs
                no matmul and no psum priming)."""
                if b >= nblk or b in csts:
                    return
                cs = cbp.tile([128, BLK * 128], F16, tag="cs")
                csts[b] = cs
                # cst col = s*256 + g*64 + x (x = 2c+e), iterated (g, s, x)
                cs4 = cs[:].rearrange(
                    "p (s g x) -> p g s x", s=SB, g=4, x=64)
                if b >= SKEW // BLK:
                    # ONE merged 4-bank drain + per-partition fc bias; the
                    # conv ones-row weights pre-subtract fcb[p] host-side so
                    # the m1 banks come out exact
                    nc.scalar.activation(
                        out=cs4[:, :, :, :],
                        in_=chs[b][:].rearrange(
                            "p (g s x) -> p g s x", g=4, s=SB, x=64),
                        func=AF.Identity, bias=fcb[:],
                    )
                else:
                    nc.scalar.activation(
                        out=cs4[:, 0:G, :, :],
                        in_=chs[b][:, 0:G * NB].rearrange(
                            "p (g s x) -> p g s x", g=G, s=SB, x=64),
                        func=AF.Identity, bias=fcb[:],
                    )
                    nc.vector.memset(cs4[:, G, :, :], 0.0)

            def spikes_and_fc(b):
                """After block b's ticks: sigma = (m1 >= 1) in {0,1} fp16
                (one DVE stock tensor_scalar reading packed (c,e) pairs so
                the fw 2x/4x perf modes stay eligible), then fc (3 fp16
                matmuls, start=True on g0) into C tile b+SKEW/BLK."""
                lead = SKEW // BLK
                if b < 0 or b + lead >= nblk:
                    return
                # (c, e) pairs are contiguous 64-element runs in both ring
                # and sigma layouts — coalesce so the fw 4x perf mode holds;
                # iterate time-major (s, g, x) like the fast pre-pair layout
                ring4s = ring_of(b).rearrange(
                    "p (s g x) -> p s g x", s=SB, g=4, x=64)
                sg = sgp.tile([128, G * NB], F16, tag="sg")
                sg4 = sg[:].rearrange(
                    "p (g s x) -> p s g x", g=G, s=SB, x=64)
                nc.vector.tensor_scalar(
                    out=sg4, in0=ring4s[:, :, 0:G, :],
                    scalar1=THR, scalar2=None, op0=ALU.is_ge,
                )
                for g in range(G):
                    nc.tensor.matmul(
                        out=chs[b + lead][0:J, G * NB:4 * NB],
                        lhsT=fch[:, g * J:(g + 1) * J],
                        rhs=sg[:, g * NB:(g + 1) * NB],
                        start=(g == 0), stop=(g == G - 1),
                        skip_group_check=True,
                    )

            def hist_dma(b):
                """mem2 of DVE-tick block b = m2 ticks [16b-SKEW, ...):
                DMA straight from the fp16 ring to DRAM (host sums).
                hist col stays t*BC + c; ring src is (s, c, e)."""
                t0 = b * BLK - SKEW
                if t0 < 0:
                    return
                n = min(BLK, T - t0)
                if n <= 0:
                    return
                # hist keeps the ring's native pair order: global col =
                # 64*(t0/2 + s) + 2c + e; the host sums over (s, e) anyway
                ring3 = ring_of(b).rearrange(
                    "p (s x) -> p s x", s=SB, x=SP)
                dst3 = hist_d[:, t0 * BC:(t0 + n) * BC].rearrange(
                    "j (s x) -> j s x", s=n // 2, x=64)
                # issue from the (idle) Pool engine so hist DMAs don't queue
                # behind the x-window loads on the sync engine's DMA queues
                nc.gpsimd.dma_start(
                    dst3, ring3[0:J, 0:n // 2, 192:256],
                )

            # prologue: drive pipeline primed one block deep
            load_window(0)
            load_window(1)
            ensure_psum(0)
            ensure_psum(1)
            conv_copies(0)
            conv_copies(1)

            for b in range(nblk):
                load_window((b * BLK) // WIN + 2)
                ensure_psum(b + 2)
                # lead=5: use sigma from THREE iterations ago so the fc
                # matmuls never gate the psum drain, and drain a full iter
                # ahead of the LIF consumer
                spikes_and_fc(b - 3)
                conv_copies(b + 2)
                hist_dma(b - 1)

                base = (b % NR) * RNG
                cst = csts[b]
                nt = min(BLK, ticks - b * BLK)      # ticks in this block
                ns = nt // 2                        # super-ticks (nt is even)
                if b % NR == 0:
                    # wrap: super 0's predecessor is the END of the region
                    nc.vector._custom_dve(
                        dbl,
                        out=bigring[:, base:base + SP],
                        in0=bigring[:, NR * RNG - SP:NR * RNG],
                        in1=cst[:, 0:SP],
                        s0=BETA, s1=THR,
                    )
                    if ns > 1:
                        nc.vector._custom_dve(
                            dbl,
                            out=bigring[:, base + SP:base + ns * SP],
                            in0=bigring[:, base:base + (ns - 1) * SP],
                            in1=cst[:, SP:ns * SP],
                            s0=BETA, s1=THR,
                        )
                else:
                    # predecessor slot is contiguous below: ONE op covers
                    # all supers; in0 = own out shifted one super back,
                    # crossing the slot boundary (128-pair RAW distance
                    # within the op; the interleaved sign op spaces the
                    # cross-instruction boundary)
                    nc.vector._custom_dve(
                        dbl,
                        out=bigring[:, base:base + ns * SP],
                        in0=bigring[:, base - SP:base + (ns - 1) * SP],
                        in1=cst[:, 0:ns * SP],
                        s0=BETA, s1=THR,
                    )
            # epilogue: the last block's mem2 history
            hist_dma(nblk - 1)

    # the Tile scheduling pass re-emits instructions, so the perf-mode
    # request must be stamped on the FINAL instruction list (byte-36 bits
    # 7:6). The 2X_1PORT program IS the double-step semantics — required,
    # not optional (the REGULAR slot is a canary).
    for bb in nc.main_func.blocks:
        for i in bb.instructions:
            if (type(i).__name__ == "InstCustomDveAnt"
                    and i.op_name == dbl.name):
                i.perf_max = 1
    nc.compile()
    return nc


def _bf16_split(a):
    import ml_dtypes
    hi = a.astype(ml_dtypes.bfloat16)
    lo = (a - hi.astype(np.float32)).astype(ml_dtypes.bfloat16)
    return hi, lo


def _host_prep(x, conv_w, conv_b, fc_w, fc_b, T):
    """Build per-core input maps (numpy only)."""
    ticks = T + SKEW
    nblk = -(-ticks // BLK)
    windows = -(-(nblk * BLK) // WIN)
    xt_ticks = windows * WIN

    fcb = np.zeros((128, 1), np.float32)
    fcb[:J, 0] = fc_b

    wexp = np.zeros((KX, F), np.float32)
    for c in range(CH):
        for l in range(LO):
            wexp[l:l + 7, c * LO + l] = conv_w[c, 0, :]
        wexp[L_IN, c * LO:(c + 1) * LO] = conv_b[c]
    # the merged ACT drain adds fcb[p] to EVERY partition; pre-subtract it
    # from the conv ones-row so the m1 drive stays exact (feature f lands on
    # partition f % 128)
    for f in range(F):
        wexp[L_IN, f] -= fcb[f % 128, 0]
    weh, wel = _bf16_split(wexp)
    wes = np.concatenate([weh, weh, wel], axis=0)  # K-stacked [93, F]

    # spikes s = (m1 >= 1) in {0,1}: c2 = fc_w @ s + fc_b with plain fp16
    # weights; the bias rides the ACT fc-bank drain's per-partition bias port
    fcwt = np.zeros((128, G * J), np.float32)
    for g in range(G):
        fcwt[:, g * J:(g + 1) * J] = fc_w[:, g * 128:(g + 1) * 128].T
    fch = fcwt.astype(np.float16)

    in_maps = []
    B = x.shape[0]
    n_cores = B // BC
    for core in range(n_cores):
        xc = x[core * BC:(core + 1) * BC]          # [BC, T, L]
        xt = np.zeros((KX, xt_ticks, BC), np.float32)
        xt[:L_IN, :T, :] = xc.transpose(2, 1, 0)
        xt[L_IN, :T, :] = 1.0
        # pair-interleave: col = s*64 + 2c + e with tick = 2s + e
        xt = (xt.reshape(KX, xt_ticks // 2, 2, BC)
                .transpose(0, 1, 3, 2)
                .reshape(KX, xt_ticks * BC))
        xth, xtl = _bf16_split(xt)
        xstk = np.concatenate([xth, xtl, xth], axis=0)  # [93, cols]
        in_maps.append({
            "xts": xstk, "wes": wes, "fch": fch, "fcb": fcb,
        })
    return in_maps


def _install_trace_hook():
    """Wire up the axon NTFF profiling hook (absent from this image)."""
    import types

    if "antenv.axon_hooks" in sys.modules:
        return True
    try:
        if "/root/.axon_site" not in sys.path:
            sys.path.insert(0, "/root/.axon_site")
        from trn_agent_boot.trn_boot import _ntff_profile_via_ctypes

        hook = _ntff_profile_via_ctypes("/opt/axon/libaxon_pjrt.so")
        if hook is None:
            return False
        mod = types.ModuleType("antenv.axon_hooks")
        mod.get_axon_ntff_profile_hook = lambda: hook
        sys.modules["antenv.axon_hooks"] = mod
        import concourse.bass_utils as bu

        bu.upload_artifacts = lambda tmpdir: str(tmpdir)
        return True
    except Exception as e:  # profiling is optional
        print(f"trace hook install failed: {e}", file=sys.stderr)
        return False


def run_cores(x, conv_w, conv_b, fc_w, fc_b, T=None):
    """Run the Bass kernel on len(batch)/32 cores; returns [B, 35] output."""
    global LAST_RESULTS
    T = T if T is not None else x.shape[1]
    trace = TRACE and _install_trace_hook()
    nc = _build_nc(T)
    in_maps = _host_prep(x, conv_w, conv_b, fc_w, fc_b, T)
    res = run_bass_kernel_spmd(
        nc, in_maps, core_ids=list(range(len(in_maps))), trace=trace,
    )
    LAST_RESULTS = res
    outs = []
    for i in range(len(in_maps)):
        hv = np.asarray(res.results[i]["hist"], dtype=np.float32)
        # pair-interleaved: col = 64*s + 2c + e -> [J, T/2, sample, parity]
        m2 = hv.reshape(J, T // 2, BC, 2)
        outs.append((m2.sum(axis=(1, 3)) / np.float32(T)).T.astype(np.float32))
    return np.concatenate(outs, axis=0)


def kernel(x, conv_w, conv_b, fc_w, fc_b):
    return run_cores(
        np.asarray(x, np.float32), np.asarray(conv_w, np.float32),
        np.asarray(conv_b, np.float32), np.asarray(fc_w, np.float32),
        np.asarray(fc_b, np.float32),
    )


# revision 56
# speedup vs baseline: 1.0515x; 1.0078x over previous
"""Trainium2 Bass kernel for nn_BasicClassifier (spiking conv classifier).

Sharding: pure data parallelism — batch 256 is split 32 samples per core
across 8 NeuronCores; params are replicated (tiny).

Per-core design (~154us). The T=1000 LIF scan is sequential; everything is
column-ordered PAIR-INTERLEAVED by (super-tick s, sample c, parity e) with
tick = 2s+e so the DVE can run a custom 2X_1PORT "double-step" op:

  - State ring: fp16 [128, 4*2048] contiguous region of 4 block slots.
    Within a slot, col s*256 + 2i + e holds
    membrane i at tick 2s+e (i<96: layer-1 feature g*32+c; i>=96: layer-2
    unit row x sample col), lagged SKEW=80 ticks for layer 2.
  - LIF_DBL_ANT59: hand-written 2X_1PORT uOp program computing TWO LIF
    steps  m' = (m*0.9 + c) - (m > 1)  per cycle-column: reads the in0
    pair (HI = prev odd-tick state), the in1 drive pair (c_even, c_odd),
    chains both steps through the 8 ALU stages, writes the (m_e, m_o) pair
    via WR0_LO/HI. Recurrence lag = 128 pairs = 128 cycles of write->read
    distance (measured safe; 64 cycles is NOT — see pitch sweep). The 4
    ring slots live in ONE contiguous SBUF region, so a block is ONE fused
    op whose in0 reads across the slot boundary; only every 4th block (the
    wrap) needs a bridge. perf_max=1 must be stamped on the FINAL
    instruction list (the Tile scheduler re-emits instructions).
  - PE: 3 conv matmuls (bf16 [xh;xl;xh]x[wh;wh;wl] K-stacks, exact to
    ~2^-16) + 3 fc matmuls (fp16 weights over sigma=(m1>=1) in {0,1},
    start=True on g0) per block. No bias matmul anywhere.
  - ACT: ONE merged 4-bank psum drain into the fp16 drive tile with the fc
    bias added via the per-partition bias port; the conv ones-row weights
    pre-subtract fcb[f%128] host-side so the m1 banks stay exact.
  - DVE stock tensor_scalar(is_ge) makes sigma (runs the fw 2x/4x modes).
  - fc sigma lead = 3 blocks (SKEW=5*BLK) so the fc matmuls never gate
    the psum drain, which runs a full iteration ahead of its consumer.
  - mem2 history: DMA from ring cols (s,192:256) via the POOL engine's DMA
    queues (latency-critical ring WAR; keeps them off the x-window queue).
    Host sums over (s, e).
"""

import os
import sys

for _p in ("/opt/trn_rl_repo", "/opt/pypackages"):
    if _p not in sys.path:
        sys.path.insert(0, _p)

import numpy as np

import concourse.bacc as bacc
import concourse.mybir as mybir
import concourse.tile as tile
import concourse.dve_ops as dve_ops
from concourse.dve_spec import Spec, Src0, Src1, C0, C1, lower
from concourse.dve_uop import (
    AluInp,
    AluOp,
    DelayInp,
    DveOpSpec,
    InpSel,
    OutPath,
    OutSel,
    Trigger,
    UopConfig,
)
from concourse.bass_utils import run_bass_kernel_spmd

F32 = mybir.dt.float32
F16 = mybir.dt.float16
BF16 = mybir.dt.bfloat16
ALU = mybir.AluOpType
AF = mybir.ActivationFunctionType

N_CORES = 8
B_FULL, T_FULL, L_IN = 256, 1000, 30
BC = B_FULL // N_CORES      # 32 samples per core
CH, LO = 16, 24
F = CH * LO                 # 384 features
G = 3                       # feature groups of 128
J = 35                      # fc outputs
KX = L_IN + 1               # conv contraction rows (30 taps + ones row)
BLK = 16                    # ticks per block (= 4 PSUM banks of drive)
SKEW = 5 * BLK              # layer-2 lag: spikes at tick t drive m2 at t+SKEW
WIN = 160                   # ticks per x-window DMA (multiple of BLK)
BETA, THR = 0.9, 1.0

TRACE = bool(int(os.environ.get("KERNEL_TRACE", "0")))
LIF2X = bool(int(os.environ.get("KERNEL_LIF2X", "1")))
LAST_RESULTS = None

_LIF_OP = None
_DBL_OP = None


def _lif_dbl_2x_uop():
    """2X_1PORT uOp program computing a DOUBLE LIF step per cycle.

    State is stored pair-interleaved: ring col s*256 + 2i + e holds membrane
    i at tick 2s+e. Each cycle the engine reads one in0 pair (only the HI
    element m = tick 2s-1 state is used), one in1 pair (c_even, c_odd), and
    the 8 ALU stages chain two full LIF steps, emitting the (m_even, m_odd)
    pair via WR0_LO/WR0_HI. The recurrence lag is one super-tick = 128
    pairs = 128 cycles of write->read distance (the same margin the proven
    1x single-step design has)."""
    u = UopConfig()
    u.enable_input(InpSel.SRC_0_HI, 1)   # d0 = m (prev odd-tick state)
    u.enable_input(InpSel.SRC_1, 2)      # d1 = c_even
    u.enable_input(InpSel.SRC_1_HI, 3)   # d2 = c_odd
    u.enable_input(InpSel.CONST_0, 4)    # d3 = beta
    u.enable_input(InpSel.CONST_1, 5)    # d4 = thr
    u.require_inp0 = 1
    u.require_inp1 = 1
    u.trigger = (Trigger.SRC_TENSOR_DONE, Trigger.NONE, Trigger.NONE)
    u.next_uop = (0, 0, 0)
    dp = u.datapath_config
    # S0: alu = m*beta
    dp[0].enable_alu(AluOp.MULTIPLY, AluInp.PREV_DELAY_0, AluInp.PREV_DELAY_3
                     ).pass_through_delay(0, 1, 2, 3, 4)
    # S1: alu = f_a = m*beta + c_even
    dp[1].enable_alu(AluOp.ADD, AluInp.PREV_ALU_OUT, AluInp.PREV_DELAY_1
                     ).pass_through_delay(0, 2, 3, 4)
    # S2: alu = H_a = (thr < m); d1 <- f_a
    dp[2].enable_alu(AluOp.IS_LT, AluInp.PREV_DELAY_4, AluInp.PREV_DELAY_0
                     ).pass_through_delay(2, 3, 4
                     ).enable_delay_from_src(DelayInp.PREV_ALU_OUT, 1)
    # S3: alu = m_a = f_a - H_a
    dp[3].enable_alu(AluOp.SUBTRACT, AluInp.PREV_DELAY_1, AluInp.PREV_ALU_OUT
                     ).pass_through_delay(2, 3, 4)
    # S4: alu = m_a*beta; d0 <- m_a
    dp[4].enable_alu(AluOp.MULTIPLY, AluInp.PREV_ALU_OUT, AluInp.PREV_DELAY_3
                     ).pass_through_delay(2, 4
                     ).enable_delay_from_src(DelayInp.PREV_ALU_OUT, 0)
    # S5: alu = f_b = m_a*beta + c_odd
    dp[5].enable_alu(AluOp.ADD, AluInp.PREV_ALU_OUT, AluInp.PREV_DELAY_2
                     ).pass_through_delay(0, 4)
    # S6: alu = H_b = (thr < m_a); d1 <- f_b
    dp[6].enable_alu(AluOp.IS_LT, AluInp.PREV_DELAY_4, AluInp.PREV_DELAY_0
                     ).pass_through_delay(0
                     ).enable_delay_from_src(DelayInp.PREV_ALU_OUT, 1)
    # S7: alu = m_b = f_b - H_b
    dp[7].enable_alu(AluOp.SUBTRACT, AluInp.PREV_DELAY_1, AluInp.PREV_ALU_OUT
                     ).pass_through_delay(0)
    u.enable_output(OutSel.DELAY_0, OutPath.WR0_LO)   # m_even
    u.enable_output(OutSel.ALU_OUT, OutPath.WR0_HI)   # m_odd
    return u


def _get_dbl_op():
    """Register the pair-interleaved double-step LIF op. The REGULAR (1x)
    variant is the plain single-step program — it is semantically WRONG for
    the pair layout and acts as a loud canary should the engine ever fall
    back (our APs always qualify for 2X_1PORT)."""
    global _DBL_OP
    if _DBL_OP is not None:
        return _DBL_OP
    name = "LIF_DBL_ANT59"
    for op in dve_ops.OPS:
        if op.name == name:
            _DBL_OP = op
            return op

    def _ref(in0, in1, s0, s1, imm2):
        a = in0.astype(np.float32)
        c = np.asarray(in1, np.float32).reshape(a.shape)
        m = a[..., 1::2]
        f_a = m * np.float32(s0) + c[..., 0::2]
        m_a = f_a - (m > s1).astype(np.float32)
        f_b = m_a * np.float32(s0) + c[..., 1::2]
        m_b = f_b - (m_a > s1).astype(np.float32)
        out = np.empty_like(a)
        out[..., 0::2] = m_a
        out[..., 1::2] = m_b
        return out.astype(np.float32)

    spec = Spec(
        body=(Src0 * C0 + Src1) - (Src0 > C1),
        reference=_ref,
    )
    row = dve_ops._CUSTOM_DVE_ROW_BASE + len(dve_ops.OPS)
    assert row < 0x20
    dve_ops._SUB_OPCODE_FOR_NAME[name] = row
    compiled = DveOpSpec(
        name=name, opcode=row, uops=lower(spec, ver="v3"), rd1_en=True,
        uops_2x=[_lif_dbl_2x_uop()], perf_max=1,
    )
    compiled.validate("v3")
    op = dve_ops.DveOp(name, spec, subdim=False,
                       uops_sha={"v3": compiled.sha("v3")})
    dve_ops.OPS.append(op)
    dve_ops.CUSTOM_DVE_SPECS[name] = spec
    dve_ops._COMPILE_CACHE[(name, "v3")] = compiled
    _DBL_OP = op
    return op


def _lif_2x_uop():
    """2X_1PORT uOp program for the LIF step: each cycle the engine reads a
    packed pair of fp16 elements per port (SRC_* = element 0, SRC_*_HI =
    element 1). Element 0's chain runs on ALU stages 0-3, element 1's on
    stages 4-7; results go out packed via WR0_LO/WR0_HI."""
    u = UopConfig()
    u.enable_input(InpSel.SRC_0, 1)      # delay0 = m_e0
    u.enable_input(InpSel.SRC_0_HI, 2)   # delay1 = m_e1
    u.enable_input(InpSel.SRC_1, 3)      # delay2 = c_e0
    u.enable_input(InpSel.SRC_1_HI, 4)   # delay3 = c_e1
    u.enable_input(InpSel.CONST_0, 5)    # delay4 = beta
    u.enable_input(InpSel.CONST_1, 6)    # delay5 = thr
    u.require_inp0 = 1
    u.require_inp1 = 1
    u.trigger = (Trigger.SRC_TENSOR_DONE, Trigger.NONE, Trigger.NONE)
    u.next_uop = (0, 0, 0)
    dp = u.datapath_config
    # S0: alu = m0*beta
    dp[0].enable_alu(AluOp.MULTIPLY, AluInp.PREV_DELAY_0, AluInp.PREV_DELAY_4
                     ).pass_through_delay(0, 1, 2, 3, 4, 5)
    # S1: alu = f0 = m0*beta + c0
    dp[1].enable_alu(AluOp.ADD, AluInp.PREV_ALU_OUT, AluInp.PREV_DELAY_2
                     ).pass_through_delay(0, 1, 3, 4, 5)
    # S2: alu = H0 = (thr < m0); d2 <- f0
    dp[2].enable_alu(AluOp.IS_LT, AluInp.PREV_DELAY_5, AluInp.PREV_DELAY_0
                     ).pass_through_delay(1, 3, 4, 5
                     ).enable_delay_from_src(DelayInp.PREV_ALU_OUT, 2)
    # S3: alu = out0 = f0 - H0
    dp[3].enable_alu(AluOp.SUBTRACT, AluInp.PREV_DELAY_2, AluInp.PREV_ALU_OUT
                     ).pass_through_delay(1, 3, 4, 5)
    # S4: alu = m1*beta; d0 <- out0
    dp[4].enable_alu(AluOp.MULTIPLY, AluInp.PREV_DELAY_1, AluInp.PREV_DELAY_4
                     ).pass_through_delay(1, 3, 5
                     ).enable_delay_from_src(DelayInp.PREV_ALU_OUT, 0)
    # S5: alu = f1 = m1*beta + c1
    dp[5].enable_alu(AluOp.ADD, AluInp.PREV_ALU_OUT, AluInp.PREV_DELAY_3
                     ).pass_through_delay(0, 1, 5)
    # S6: alu = H1 = (thr < m1); d2 <- f1
    dp[6].enable_alu(AluOp.IS_LT, AluInp.PREV_DELAY_5, AluInp.PREV_DELAY_1
                     ).pass_through_delay(0
                     ).enable_delay_from_src(DelayInp.PREV_ALU_OUT, 2)
    # S7: alu = out1 = f1 - H1
    dp[7].enable_alu(AluOp.SUBTRACT, AluInp.PREV_DELAY_2, AluInp.PREV_ALU_OUT
                     ).pass_through_delay(0)
    u.enable_output(OutSel.DELAY_0, OutPath.WR0_LO)   # out0
    u.enable_output(OutSel.ALU_OUT, OutPath.WR0_HI)   # out1
    return u


def _get_lif_op():
    """Register the fused LIF-step op in the custom-DVE table (idempotent)."""
    global _LIF_OP
    if _LIF_OP is not None:
        return _LIF_OP
    name = "LIF_STEP_ANT59"
    for op in dve_ops.OPS:
        if op.name == name:
            _LIF_OP = op
            return op
    spec = Spec(
        body=(Src0 * C0 + Src1) - (Src0 > C1),
        reference=lambda in0, in1, s0, s1, imm2: (
            (in0.astype(np.float32) * np.float32(s0)
             + in1.reshape(in0.shape))
            - (in0 > s1).astype(np.float32)
        ).astype(np.float32),
    )
    row = dve_ops._CUSTOM_DVE_ROW_BASE + len(dve_ops.OPS)
    assert row < 0x20
    dve_ops._SUB_OPCODE_FOR_NAME[name] = row
    compiled = DveOpSpec(
        name=name, opcode=row, uops=lower(spec, ver="v3"), rd1_en=True,
        uops_2x=[_lif_2x_uop()] if LIF2X else None,
        perf_max=1 if LIF2X else 0,
    )
    compiled.validate("v3")
    op = dve_ops.DveOp(name, spec, subdim=False,
                       uops_sha={"v3": compiled.sha("v3")})
    dve_ops.OPS.append(op)
    dve_ops.CUSTOM_DVE_SPECS[name] = spec
    dve_ops._COMPILE_CACHE[(name, "v3")] = compiled
    _LIF_OP = op
    return op


def _build_nc(T):
    """Build the per-core Bass program (SPMD: same program on every core).

    Pair-interleaved layout: everything column-ordered by (super s, sample c,
    parity e) with tick = 2s+e. Ring/drive col = s*256 + 2i + e (state index
    i: m1 i=g*32+c, m2 i=96+c); sigma col = g*512 + s*64 + 2c + e; psum bank
    col = s*64 + 2c + e; host orders the conv rhs columns the same way."""
    dbl = _get_dbl_op()
    ticks = T + SKEW                       # DVE ticks 0..T+SKEW-1
    nblk = -(-ticks // BLK)
    pad_ticks = nblk * BLK
    windows = -(-pad_ticks // WIN)
    xt_cols = windows * WIN * BC
    NB = BLK * BC                          # 512 sample-ticks per block
    SB = BLK // 2                          # 8 super-ticks per block
    SP = 256                               # ring cols per super-tick

    nc = bacc.Bacc("TRN2", target_bir_lowering=False)

    KS = 3 * KX                            # stacked conv K: [xh; xl; xh]
    xts_d = nc.dram_tensor("xts", [KS, xt_cols], BF16, kind="ExternalInput")
    wes_d = nc.dram_tensor("wes", [KS, F], BF16, kind="ExternalInput")
    fch_d = nc.dram_tensor("fch", [128, G * J], F16, kind="ExternalInput")
    fcb_d = nc.dram_tensor("fcb", [128, 1], F32, kind="ExternalInput")
    hist_d = nc.dram_tensor("hist", [J, BC * T], F16, kind="ExternalOutput")

    with tile.TileContext(nc) as tc:
        with (
            tc.tile_pool(name="konst", bufs=1) as kp,
            tc.tile_pool(name="ring", bufs=1) as rp,
            tc.tile_pool(name="sig", bufs=3) as sgp,
            tc.tile_pool(name="xwin", bufs=3) as xp,
            tc.tile_pool(name="cdrv", bufs=3) as cbp,
            tc.tile_pool(name="cpsum", bufs=2, space="PSUM") as cp,
        ):
            # constants -> SBUF
            wes = kp.tile([KS, F], BF16, tag="wes")
            fch = kp.tile([128, G * J], F16, tag="fch")
            fcb = kp.tile([128, 1], F32, tag="fcb")
            for sb, dr in ((wes, wes_d), (fch, fch_d), (fcb, fcb_d)):
                nc.sync.dma_start(sb[:], dr[:])

            # state ring: ONE contiguous fp16 region of 4 block-sized
            # slots so a block whose predecessor slot sits directly before
            # it needs NO bridge op (in0 reads straight across the slot
            # boundary); only every NR-th block (the wrap) bridges
            NR = 4
            RNG = BLK * 128
            bigring = rp.tile([128, NR * RNG], F16, tag="bigring")
            # only block 0's wrap-bridge reads pre-existing ring state (the
            # last super of the region); everything else is written first
            nc.vector.memset(bigring[:, NR * RNG - SP:], 0.0)

            def ring_of(b):
                return bigring[:, (b % NR) * RNG:((b % NR) + 1) * RNG]

            xts = {}      # window idx -> xt sbuf tile
            chs = {}      # block idx -> PSUM C tile [128, 4*512] bank-major
            csts = {}     # block idx -> SBUF fp32 drive tile, tick-major

            def load_window(w):
                # chunked into 10 DMAs so latency-critical hist DMAs behind
                # them on the same queues wait ~1us, not the whole window
                if w < 0 or w >= windows or w in xts:
                    return
                ts = xp.tile([KS, WIN * BC], BF16, tag="xws")
                step = WIN * BC // 10
                for i in range(10):
                    nc.sync.dma_start(
                        ts[:, i * step:(i + 1) * step],
                        xts_d[:, w * WIN * BC + i * step:w * WIN * BC + (i + 1) * step],
                    )
                xts[w] = ts

            def ensure_psum(b):
                """Allocate block b's bank-major PSUM C tile (banks 0-2 conv,
                bank 3 fc; psum col = g*512 + t*32 + c)."""
                if b >= nblk or b in chs:
                    return
                ch = cp.tile([128, 4 * NB], F32, tag="ch")
                chs[b] = ch
                if b >= nblk - SKEW // BLK:
                    # tail: this block's m1 is never consumed (its sigma
                    # would target past-the-end blocks) — skip the conv
                    return
                w = (b * BLK) // WIN
                base = (b * BLK - w * WIN) * BC
                for g in range(G):
                    nc.tensor.matmul(
                        out=ch[:, g * NB:(g + 1) * NB],
                        lhsT=wes[:, g * 128:(g + 1) * 128],
                        rhs=xts[w][:, base:base + NB],
                        start=True, stop=True,
                    )

            def conv_copies(b):
                """ACT: op1 drains the 3 conv psum banks into the
                pair-interleaved fp16 SBUF drive tile; op2 drains the fc
                bank ADDING the per-partition fc bias (so the fc bias needs
                no matmul and no psum priming)."""
                if b >= nblk or b in csts:
                    return
                cs = cbp.tile([128, BLK * 128], F16, tag="cs")
                csts[b] = cs
                # cst col = s*256 + g*64 + x (x = 2c+e), iterated (g, s, x)
                cs4 = cs[:].rearrange(
                    "p (s g x) -> p g s x", s=SB, g=4, x=64)
                if b >= nblk - SKEW // BLK:
                    # tail: only the fc bank matters (m1 drive is dead);
                    # the cst m1 region keeps stale-but-finite values
                    nc.scalar.activation(
                        out=cs4[:, G, :, :],
                        in_=chs[b][:, G * NB:4 * NB].rearrange(
                            "p (s x) -> p s x", s=SB, x=64),
                        func=AF.Identity, bias=fcb[:],
                    )
                elif b >= SKEW // BLK:
                    # ONE merged 4-bank drain + per-partition fc bias; the
                    # conv ones-row weights pre-subtract fcb[p] host-side so
                    # the m1 banks come out exact
                    nc.scalar.activation(
                        out=cs4[:, :, :, :],
                        in_=chs[b][:].rearrange(
                            "p (g s x) -> p g s x", g=4, s=SB, x=64),
                        func=AF.Identity, bias=fcb[:],
                    )
                else:
                    nc.scalar.activation(
                        out=cs4[:, 0:G, :, :],
                        in_=chs[b][:, 0:G * NB].rearrange(
                            "p (g s x) -> p g s x", g=G, s=SB, x=64),
                        func=AF.Identity, bias=fcb[:],
                    )
                    nc.vector.memset(cs4[:, G, :, :], 0.0)

            def spikes_and_fc(b):
                """After block b's ticks: sigma = (m1 >= 1) in {0,1} fp16
                (one DVE stock tensor_scalar reading packed (c,e) pairs so
                the fw 2x/4x perf modes stay eligible), then fc (3 fp16
                matmuls, start=True on g0) into C tile b+SKEW/BLK."""
                lead = SKEW // BLK
                if b < 0 or b + lead >= nblk:
                    return
                # (c, e) pairs are contiguous 64-element runs in both ring
                # and sigma layouts — coalesce so the fw 4x perf mode holds;
                # iterate time-major (s, g, x) like the fast pre-pair layout
                ring4s = ring_of(b).rearrange(
                    "p (s g x) -> p s g x", s=SB, g=4, x=64)
                sg = sgp.tile([128, G * NB], F16, tag="sg")
                sg4 = sg[:].rearrange(
                    "p (g s x) -> p s g x", g=G, s=SB, x=64)
                nc.vector.tensor_scalar(
                    out=sg4, in0=ring4s[:, :, 0:G, :],
                    scalar1=THR, scalar2=None, op0=ALU.is_ge,
                )
                for g in range(G):
                    nc.tensor.matmul(
                        out=chs[b + lead][0:J, G * NB:4 * NB],
                        lhsT=fch[:, g * J:(g + 1) * J],
                        rhs=sg[:, g * NB:(g + 1) * NB],
                        start=(g == 0), stop=(g == G - 1),
                        skip_group_check=True,
                    )

            def hist_dma(b):
                """mem2 of DVE-tick block b = m2 ticks [16b-SKEW, ...):
                DMA straight from the fp16 ring to DRAM (host sums).
                hist col stays t*BC + c; ring src is (s, c, e)."""
                t0 = b * BLK - SKEW
                if t0 < 0:
                    return
                n = min(BLK, T - t0)
                if n <= 0:
                    return
                # hist keeps the ring's native pair order: global col =
                # 64*(t0/2 + s) + 2c + e; the host sums over (s, e) anyway
                ring3 = ring_of(b).rearrange(
                    "p (s x) -> p s x", s=SB, x=SP)
                dst3 = hist_d[:, t0 * BC:(t0 + n) * BC].rearrange(
                    "j (s x) -> j s x", s=n // 2, x=64)
                # issue from the (idle) Pool engine so hist DMAs don't queue
                # behind the x-window loads on the sync engine's DMA queues
                nc.gpsimd.dma_start(
                    dst3, ring3[0:J, 0:n // 2, 192:256],
                )

            # prologue: drive pipeline primed one block deep
            load_window(0)
            load_window(1)
            ensure_psum(0)
            ensure_psum(1)
            conv_copies(0)
            conv_copies(1)

            for b in range(nblk):
                load_window((b * BLK) // WIN + 2)
                ensure_psum(b + 2)
                # lead=5: use sigma from THREE iterations ago so the fc
                # matmuls never gate the psum drain, and drain a full iter
                # ahead of the LIF consumer
                spikes_and_fc(b - 3)
                conv_copies(b + 2)
                hist_dma(b - 1)

                base = (b % NR) * RNG
                cst = csts[b]
                nt = min(BLK, ticks - b * BLK)      # ticks in this block
                ns = nt // 2                        # super-ticks (nt is even)
                if b % NR == 0:
                    # wrap: super 0's predecessor is the END of the region
                    nc.vector._custom_dve(
                        dbl,
                        out=bigring[:, base:base + SP],
                        in0=bigring[:, NR * RNG - SP:NR * RNG],
                        in1=cst[:, 0:SP],
                        s0=BETA, s1=THR,
                    )
                    if ns > 1:
                        nc.vector._custom_dve(
                            dbl,
                            out=bigring[:, base + SP:base + ns * SP],
                            in0=bigring[:, base:base + (ns - 1) * SP],
                            in1=cst[:, SP:ns * SP],
                            s0=BETA, s1=THR,
                        )
                else:
                    # predecessor slot is contiguous below: ONE op covers
                    # all supers; in0 = own out shifted one super back,
                    # crossing the slot boundary (128-pair RAW distance
                    # within the op; the interleaved sign op spaces the
                    # cross-instruction boundary)
                    nc.vector._custom_dve(
                        dbl,
                        out=bigring[:, base:base + ns * SP],
                        in0=bigring[:, base - SP:base + (ns - 1) * SP],
                        in1=cst[:, 0:ns * SP],
                        s0=BETA, s1=THR,
                    )
            # epilogue: the last block's mem2 history
            hist_dma(nblk - 1)

    # the Tile scheduling pass re-emits instructions, so the perf-mode
    # request must be stamped on the FINAL instruction list (byte-36 bits
    # 7:6). The 2X_1PORT program IS the double-step semantics — required,
    # not optional (the REGULAR slot is a canary).
    for bb in nc.main_func.blocks:
        for i in bb.instructions:
            if (type(i).__name__ == "InstCustomDveAnt"
                    and i.op_name == dbl.name):
                i.perf_max = 1
    nc.compile()
    return nc


def _bf16_split(a):
    import ml_dtypes
    hi = a.astype(ml_dtypes.bfloat16)
    lo = (a - hi.astype(np.float32)).astype(ml_dtypes.bfloat16)
    return hi, lo


def _host_prep(x, conv_w, conv_b, fc_w, fc_b, T):
    """Build per-core input maps (numpy only)."""
    ticks = T + SKEW
    nblk = -(-ticks // BLK)
    windows = -(-(nblk * BLK) // WIN)
    xt_ticks = windows * WIN

    fcb = np.zeros((128, 1), np.float32)
    fcb[:J, 0] = fc_b

    wexp = np.zeros((KX, F), np.float32)
    for c in range(CH):
        for l in range(LO):
            wexp[l:l + 7, c * LO + l] = conv_w[c, 0, :]
        wexp[L_IN, c * LO:(c + 1) * LO] = conv_b[c]
    # the merged ACT drain adds fcb[p] to EVERY partition; pre-subtract it
    # from the conv ones-row so the m1 drive stays exact (feature f lands on
    # partition f % 128)
    for f in range(F):
        wexp[L_IN, f] -= fcb[f % 128, 0]
    weh, wel = _bf16_split(wexp)
    wes = np.concatenate([weh, weh, wel], axis=0)  # K-stacked [93, F]

    # spikes s = (m1 >= 1) in {0,1}: c2 = fc_w @ s + fc_b with plain fp16
    # weights; the bias rides the ACT fc-bank drain's per-partition bias port
    fcwt = np.zeros((128, G * J), np.float32)
    for g in range(G):
        fcwt[:, g * J:(g + 1) * J] = fc_w[:, g * 128:(g + 1) * 128].T
    fch = fcwt.astype(np.float16)

    in_maps = []
    B = x.shape[0]
    n_cores = B // BC
    for core in range(n_cores):
        xc = x[core * BC:(core + 1) * BC]          # [BC, T, L]
        xt = np.zeros((KX, xt_ticks, BC), np.float32)
        xt[:L_IN, :T, :] = xc.transpose(2, 1, 0)
        xt[L_IN, :T, :] = 1.0
        # pair-interleave: col = s*64 + 2c + e with tick = 2s + e
        xt = (xt.reshape(KX, xt_ticks // 2, 2, BC)
                .transpose(0, 1, 3, 2)
                .reshape(KX, xt_ticks * BC))
        xth, xtl = _bf16_split(xt)
        xstk = np.concatenate([xth, xtl, xth], axis=0)  # [93, cols]
        in_maps.append({
            "xts": xstk, "wes": wes, "fch": fch, "fcb": fcb,
        })
    return in_maps


def _install_trace_hook():
    """Wire up the axon NTFF profiling hook (absent from this image)."""
    import types

    if "antenv.axon_hooks" in sys.modules:
        return True
    try:
        if "/root/.axon_site" not in sys.path:
            sys.path.insert(0, "/root/.axon_site")
        from trn_agent_boot.trn_boot import _ntff_profile_via_ctypes

        hook = _ntff_profile_via_ctypes("/opt/axon/libaxon_pjrt.so")
        if hook is None:
            return False
        mod = types.ModuleType("antenv.axon_hooks")
        mod.get_axon_ntff_profile_hook = lambda: hook
        sys.modules["antenv.axon_hooks"] = mod
        import concourse.bass_utils as bu

        bu.upload_artifacts = lambda tmpdir: str(tmpdir)
        return True
    except Exception as e:  # profiling is optional
        print(f"trace hook install failed: {e}", file=sys.stderr)
        return False


def run_cores(x, conv_w, conv_b, fc_w, fc_b, T=None):
    """Run the Bass kernel on len(batch)/32 cores; returns [B, 35] output."""
    global LAST_RESULTS
    T = T if T is not None else x.shape[1]
    trace = TRACE and _install_trace_hook()
    nc = _build_nc(T)
    in_maps = _host_prep(x, conv_w, conv_b, fc_w, fc_b, T)
    res = run_bass_kernel_spmd(
        nc, in_maps, core_ids=list(range(len(in_maps))), trace=trace,
    )
    LAST_RESULTS = res
    outs = []
    for i in range(len(in_maps)):
        hv = np.asarray(res.results[i]["hist"], dtype=np.float32)
        # pair-interleaved: col = 64*s + 2c + e -> [J, T/2, sample, parity]
        m2 = hv.reshape(J, T // 2, BC, 2)
        outs.append((m2.sum(axis=(1, 3)) / np.float32(T)).T.astype(np.float32))
    return np.concatenate(outs, axis=0)


def kernel(x, conv_w, conv_b, fc_w, fc_b):
    return run_cores(
        np.asarray(x, np.float32), np.asarray(conv_w, np.float32),
        np.asarray(conv_b, np.float32), np.asarray(fc_w, np.float32),
        np.asarray(fc_b, np.float32),
    )
